# revision 24
# baseline (speedup 1.0000x reference)
"""Distributed causal multi-head attention for one TRN2 chip (8 NeuronCores).

Sharding: batch (2) x head-groups (4 heads/core) -> 8 cores.
Core c handles batch c//4, heads [ (c%4)*4 , (c%4)*4+4 ).
Per core: QKV projections for its 4 heads, flash-style causal attention
with scores kept transposed (S^T = K @ Q^T) so the PV product needs no
transposes; V is augmented with a ones column so the softmax denominators
fall out of the same matmul (row 64 of each head's O^T psum).  Then an
AllGather of the attention output (pre-Wo, 4-core group = one batch) and
a column-sliced output projection.  Host assembles the 8 column/batch
shards.  Compute dtype bf16 (PSUM accumulation fp32), softmax in fp32.

Scheduling: the attention inner loop is software-pipelined one slot deep
(S^T of slot t+1 before PV of slot t) with a FINE-GRAINED work queue:
QKV-projection and output-projection matmuls are generators yielding one
matmul at a time, and two steps are popped between each slot's S and the
previous slot's PV.  The PE therefore always has ~1us of independent
work queued while the exp of the previous slot completes on ACT, which
keeps the in-order PE queue streaming (and the tensor engine at high
p-state) instead of stalling ~300-500ns per slot.  The causal mask is
applied as an identity-matmul accumulation of an additive -30000 tile
into the S psum (PE-local), removing the DVE multiply from the
S->exp->PV critical path.  Q/K/V psum evacuations ride on DVE; exp owns
ACT exclusively.  Work items carry (deadline, earliest) slot positions:
deadlines guarantee S/PV dependencies are emitted in time on the
in-order queues, earliest-gating keeps AllGather-dependent projections
from stalling the PE before their gather lands, and the last chunk's
projections are reserved for the final drain where they fill the last
AllGather's flight time.  Inputs arrive as one wide DMA per weight
(host-packed 4KB rows; wq/wk pair-major so the first matmul waits only
on a 256KB half) and the x k-tiles load in chunk-column waves over FOUR
DMA queues for chunk 0 (sync/gpsimd/scalar/vector) so the first QKV
matmul starts ~9us in.  Softmax normalization multiplies the O^T psum
directly with the gpsimd-broadcast reciprocal of the ones-column sums;
the final chunk's AllGather bounce/tile DMAs use otherwise-idle queues.
"""

import sys
from collections import deque

import numpy as np

sys.path.insert(0, "/opt/trn_rl_repo")

import concourse.bass as bass  # noqa: E402
import concourse.bacc as bacc  # noqa: E402
import concourse.tile as tile  # noqa: E402
import concourse.mybir as mybir  # noqa: E402
from concourse.tile_rust import add_dep_helper  # noqa: E402

F32 = mybir.dt.float32
BF16 = mybir.dt.bfloat16
ActFn = mybir.ActivationFunctionType

P = 128          # partition dim
CHUNK = 512      # i-chunk (matmul moving free dim, one psum bank of fp32)
DH = 64          # head dim
HPC = 4          # heads per core
HS = HPC * DH    # 256 per-core inner slice
DHA = DH + 1     # augmented head dim (ones column for softmax sums)
INNER = 1024     # total inner dim (16 heads x 64)
N_CORES = 8
GROUPS = [[0, 1, 2, 3], [4, 5, 6, 7]]
MASKNEG = -30000.0   # additive causal mask (exp(x-30000) == 0 in fp32)


def build_nc(seq=2048, dim=1024, n_cores=N_CORES, groups=GROUPS, compile=True):
    """Build the SPMD Bass graph (identical on all cores)."""
    nch = seq // CHUNK          # i-chunks
    jpc = CHUNK // P            # j-tiles per chunk (4)
    njt = seq // P              # j-tiles
    nk = dim // P               # feature k-tiles
    nko = INNER // P            # inner k-tiles for the output projection
    grp = len(groups[0])        # replica group size (4)

    nc = bacc.Bacc("TRN2", target_bir_lowering=False, debug=False,
                   enable_asserts=False, num_devices=n_cores)

    xT = nc.dram_tensor("xT", [dim, seq], BF16, kind="ExternalInput").ap()
    # weights host-packed so each SBUF partition row is one contiguous DMA
    # row: [p, k*HS+j] = W[k*128+p, j]
    wq = nc.dram_tensor("wq", [P, nk * HS], BF16, kind="ExternalInput").ap()
    wk = nc.dram_tensor("wk", [P, nk * HS], BF16, kind="ExternalInput").ap()
    wv = nc.dram_tensor("wv", [P, nk * HS], BF16, kind="ExternalInput").ap()
    wo = nc.dram_tensor("wo", [P, nko * HS], BF16, kind="ExternalInput").ap()
    # [P, 0:2P] = additive causal mask duplicated for both heads,
    # [P, 2P:3P] = identity (mask accumulates into the S psum via matmul)
    mask_c = nc.dram_tensor("mask_c", [P, 3 * P], BF16,
                            kind="ExternalInput").ap()
    outT = nc.dram_tensor("outT", [HS, seq], BF16, kind="ExternalOutput").ap()

    with tile.TileContext(nc) as tc:
        with tc.tile_pool(name="sb", bufs=1) as sb, \
             tc.tile_pool(name="ps", bufs=1, space="PSUM") as ps, \
             tc.tile_pool(name="dram", bufs=1, space="DRAM") as dram:

            # ---- load inputs ----
            xt = [sb.tile([P, seq], BF16, tag=f"xt{k}", name=f"xt{k}")
                  for k in range(nk)]
            wq_sb = sb.tile([P, nk * HS], BF16, tag="wq", name="wq")
            wk_sb = sb.tile([P, nk * HS], BF16, tag="wk", name="wk")
            wv_sb = sb.tile([P, nk * HS], BF16, tag="wv", name="wv")
            wo_sb = sb.tile([P, nko * HS], BF16, tag="wo", name="wo")
            mask_sb = sb.tile([P, 3 * P], BF16, tag="mask", name="mask")

            # chunk-0 x wave over the three DMA-capable queues so the first
            # QKV matmul can start early; later waves on sync/gpsimd only
            # (scalar carries the wq/wk second halves, then exp owns it).
            # No CC warmup op: the collective barrier init is enqueued by
            # the runtime at a fixed ~22us regardless, and a warmup AG only
            # steals ~8us of the serial CC stream.
            q3 = [nc.sync, nc.gpsimd, nc.scalar]
            qs = [nc.sync, nc.gpsimd]

            def ld_x(k, lo, hi, queue):
                if lo < hi:
                    queue.dma_start(xt[k][:, lo:hi],
                                    xT[k * P:(k + 1) * P, lo:hi])
            hw = nk * P
            # first transfers on each queue gate the first Q matmul
            nc.scalar.dma_start(wq_sb[:, 0:hw], wq[:, 0:hw])
            for k in range(nk):
                ld_x(k, 0, CHUNK, q3[k % 3])
            nc.scalar.dma_start(wk_sb[:, 0:hw], wk[:, 0:hw])
            nc.gpsimd.dma_start(wv_sb[:], wv[:])
            nc.gpsimd.dma_start(mask_sb[:], mask_c[:])
            nc.scalar.dma_start(wq_sb[:, hw:2 * hw], wq[:, hw:2 * hw])
            nc.scalar.dma_start(wk_sb[:, hw:2 * hw], wk[:, hw:2 * hw])
            for k in range(nk):
                ld_x(k, CHUNK, min(2 * CHUNK, seq), qs[k % 2])
            nc.gpsimd.dma_start(wo_sb[:], wo[:])
            for k in range(nk):
                ld_x(k, 2 * CHUNK, seq, qs[k % 2])

            def wsl(w, k, a, b):
                return w[:, k * HS + a:k * HS + b]

            def wsl_pm(w, pair, k):
                # pair-major packed wq/wk: [p, (pair*nk + k)*128 + j]
                return w[:, (pair * nk + k) * P:(pair * nk + k + 1) * P]

            # persistent QKV results
            qt_sb = [sb.tile([P, seq], BF16, tag=f"qt{p}", name=f"qt{p}")
                     for p in range(2)]
            kt_sb = [sb.tile([P, seq], BF16, tag=f"kt{p}", name=f"kt{p}")
                     for p in range(2)]
            v_sb = [sb.tile([P, HPC * DHA], BF16, tag=f"v{j}", name=f"v{j}")
                    for j in range(njt)]
            ot_sb = [sb.tile([P, seq], BF16, tag=f"ot{p}", name=f"ot{p}")
                     for p in range(2)]
            # ones columns of the augmented V are set once up front
            for jt in range(njt):
                nc.vector.memset(
                    v_sb[jt].rearrange("p (h d) -> p h d", h=HPC)[:, :,
                                                                  DH:DHA],
                    1.0)

            # ---- work generators: one ~512-row matmul per yield ----
            def g_kt(pair, ch):
                pt = ps.tile([P, CHUNK], F32, tag="misc",
                             name=f"ktps{pair}_{ch}", bufs=2)
                for k in range(nk):
                    nc.tensor.matmul(
                        pt[:], lhsT=wsl_pm(wk_sb, pair, k),
                        rhs=xt[k][:, ch * CHUNK:(ch + 1) * CHUNK],
                        start=(k == 0), stop=(k == nk - 1))
                    if k < nk - 1:
                        yield
                nc.vector.tensor_copy(
                    kt_sb[pair][:, ch * CHUNK:(ch + 1) * CHUNK], pt[:])
                yield

            def g_qt(pair, ch):
                pt = ps.tile([P, CHUNK], F32, tag="misc",
                             name=f"qps{pair}_{ch}", bufs=2)
                for k in range(nk):
                    nc.tensor.matmul(
                        pt[:], lhsT=wsl_pm(wq_sb, pair, k),
                        rhs=xt[k][:, ch * CHUNK:(ch + 1) * CHUNK],
                        start=(k == 0), stop=(k == nk - 1))
                    if k < nk - 1:
                        yield
                nc.vector.tensor_copy(
                    qt_sb[pair][:, ch * CHUNK:(ch + 1) * CHUNK], pt[:])
                yield

            def g_v(jt):
                # N=256 matmuls: two per yield to keep step granularity even
                pt = ps.tile([P, HS], F32, tag="misc",
                             name=f"vps{jt}", bufs=2)
                for k in range(nk):
                    nc.tensor.matmul(
                        pt[:], lhsT=xt[k][:, jt * P:(jt + 1) * P],
                        rhs=wsl(wv_sb, k, 0, HS),
                        start=(k == 0), stop=(k == nk - 1))
                    if k % 2 == 1 and k < nk - 1:
                        yield
                nc.vector.tensor_copy(
                    v_sb[jt].rearrange("p (h d) -> p h d", h=HPC)[:, :, 0:DH],
                    pt.rearrange("p (h d) -> p h d", h=HPC))
                yield

            def g_proj(ci, m, slices, korder, op_box=None, evac=True,
                       loads=None):
                # transposed output block: outT[m*128:(m+1)*128, chunk ci]
                # = Wo[:, m-slice].T @ attT[:, chunk] over the k-tiles in
                # `korder` (a partial pass parks its psum in op_box).
                # `loads` issues the gathered-tile DMAs as a prologue: the
                # generator only runs once its AllGather has landed, so the
                # DMAs don't sem-block their queues.
                c0 = ci * CHUNK
                if loads:
                    # pin the gathered-tile DMAs behind the last exp: the
                    # tile scheduler's collective cost model knows nothing
                    # of the ~55us CC-stream init, and without a real edge
                    # it places these sem-blocking DMAs mid-body on the
                    # scalar queue, stalling the exp stream for ~20us
                    for q, dst, src in loads:
                        di = q.dma_start(dst, src)
                        if last_exp[0] is not None:
                            add_dep_helper(di.ins, last_exp[0],
                                           reason="ag tile load gated to drain")
                    yield
                first = op_box is None or op_box.get(m) is None
                if first:
                    op_ps = ps.tile([P, CHUNK], F32, tag="misc",
                                    name=f"op{ci}_{m}", bufs=2)
                else:
                    op_ps = op_box.pop(m)
                for n, k in enumerate(korder):
                    ag_t, coff = slices[k]
                    nc.tensor.matmul(
                        op_ps[:],
                        lhsT=wsl(wo_sb, k, m * P, (m + 1) * P),
                        rhs=ag_t[:, coff:coff + CHUNK],
                        start=(first and n == 0),
                        stop=(evac and n == len(korder) - 1))
                    if n < len(korder) - 1:
                        yield
                if not evac:
                    op_box[m] = op_ps
                    yield
                    return
                o_sb = sb.tile([P, CHUNK], BF16, tag="osb",
                               name=f"o{ci}_{m}", bufs=2)
                nc.vector.tensor_copy(o_sb[:], op_ps[:])
                nc.sync.dma_start(
                    outT[m * P:(m + 1) * P, c0:c0 + CHUNK], o_sb[:])
                yield

            # work:      [deadline, gen]  — ungated QKV work, FIFO
            # work_late: [earliest, deadline, gen] — AllGather-gated
            #            projections, FIFO; popped only once pos>=earliest
            work = deque()
            work_late = deque()
            INF = 1e9

            def drain_until(pos):
                # everything the upcoming S/PV slots depend on must already
                # be emitted on the in-order queues
                while work and work[0][0] <= pos:
                    for _ in work[0][1]:
                        pass
                    work.popleft()
                while work_late and work_late[0][1] <= pos:
                    for _ in work_late[0][2]:
                        pass
                    work_late.popleft()

            def step_work(n, pos):
                for _ in range(n):
                    if work:
                        try:
                            next(work[0][1])
                        except StopIteration:
                            work.popleft()
                    elif work_late and work_late[0][0] <= pos:
                        try:
                            next(work_late[0][2])
                        except StopIteration:
                            work_late.popleft()

            # ---- upfront: chunk-0 pass A deps emitted inline ----
            for _ in g_qt(0, 0):
                pass
            for _ in g_v(0):
                pass
            for _ in g_kt(0, 0):
                pass
            work.append([0.125, g_v(1)])
            work.append([0.25, g_v(2)])
            work.append([0.375, g_v(3)])
            work.append([0.5, g_qt(1, 0)])
            work.append([0.5, g_kt(1, 0)])

            # ---- attention chunks ----
            last_parts = {}
            last_loads = {}
            last_exp = [None]
            for ci in range(nch):
                jt_end = jpc * (ci + 1)
                c0 = ci * CHUNK
                last = ci == nch - 1

                if ci + 1 < nch:
                    njt_end = jpc * (ci + 2)
                    work.append([ci + 1, g_qt(0, ci + 1)])
                    work.append([ci + 1, g_kt(0, ci + 1)])
                    for jt in range(jpc * (ci + 1), jpc * (ci + 2)):
                        # consumed by PV(jt) in pass A of chunk ci+1
                        work.append(
                            [ci + 1 + jt / (2 * njt_end), g_v(jt)])
                    work.append([ci + 1.5, g_qt(1, ci + 1)])
                    work.append([ci + 1.5, g_kt(1, ci + 1)])

                # one bounce buffer per chunk: with bufs=2 a later chunk's
                # bounce DMA WAR-waits for an earlier AllGather to consume
                # its buffer, sem-blocking the sync queue for tens of us
                binf = None if last else dram.tile(
                    [2 * P, CHUNK], BF16, tag="binf", name=f"binf{ci}",
                    bufs=4)

                ot_ps = {}
                pend = [None]

                def do_pass_end(hp, ci=ci, c0=c0, last=last, binf=binf,
                                ot_ps=ot_ps):
                    # softmax normalize: rcp of each head's sum row (staged
                    # to SBUF), gpsimd broadcast, then one mul per head
                    # reading the O^T psum directly (no staging copy)
                    for h2 in range(2):
                        srow = sb.tile([1, CHUNK], F32, tag=f"sr{h2}",
                                       name=f"sr{ci}_{hp}_{h2}", bufs=2)
                        nc.vector.tensor_copy(srow[:],
                                              ot_ps[hp][h2][DH:DHA, :])
                        rcp = sb.tile([1, CHUNK], F32, tag=f"rcp{h2}",
                                      name=f"rcp{ci}_{hp}_{h2}", bufs=2)
                        nc.vector.reciprocal_approx_fast(rcp[:], srow[:])
                        bc_sb = sb.tile([DH, CHUNK], F32, tag=f"bc{h2}",
                                        name=f"bc{ci}_{hp}_{h2}", bufs=2)
                        nc.gpsimd.partition_broadcast(bc_sb[:], rcp[:],
                                                      channels=DH)
                        nc.vector.tensor_mul(
                            ot_sb[hp][h2 * DH:(h2 + 1) * DH, c0:c0 + CHUNK],
                            ot_ps[hp][h2][0:DH, :],
                            bc_sb[:])
                    if last:
                        tiles, loads = emit_ag_pair(ci, hp)
                        last_parts.update(tiles)
                        last_loads[hp] = loads
                    else:
                        nc.sync.dma_start(
                            binf[hp * P:(hp + 1) * P, :],
                            ot_sb[hp][:, c0:c0 + CHUNK])

                def emit_ag_full(ci, bounce_in):
                    # one AllGather for both head pairs of chunk ci (256KB;
                    # rank-major rows land so gathered row-block k*128 is
                    # exactly attT k-tile k).  Tiles are allocated here but
                    # DMA'd from the proj generator's prologue, after the
                    # gather has landed.  ch0/ch1 tiles ride sync (popped
                    # only once landed); ch2 is drain-only and its loads on
                    # gpsimd/scalar may block harmlessly there.
                    bounce_out = dram.tile([grp * 2 * P, CHUNK], BF16,
                                           tag="boutf", name=f"boutf{ci}",
                                           bufs=4)
                    nc.gpsimd.collective_compute(
                        "AllGather", mybir.AluOpType.bypass,
                        replica_groups=groups,
                        ins=[bounce_in.opt()], outs=[bounce_out.opt()])
                    tiles = {}
                    loads = []
                    for k in range(nko):
                        t = sb.tile([P, CHUNK], BF16, tag=f"ag{k}",
                                    name=f"ag{ci}_{k}", bufs=3)
                        q = nc.gpsimd if k % 2 == 0 else nc.scalar
                        loads.append(
                            (q, t[:], bounce_out[k * P:(k + 1) * P, :]))
                        tiles[k] = t
                    return tiles, loads

                def emit_ag_pair(ci, pair):
                    # half AllGather (one head pair) of the last chunk —
                    # fired right after that pair's normalize.  Gathered
                    # k-tiles land at k = 2r+pair.  Pair B's bounce rides
                    # the then-idle scalar queue; tile DMAs are issued from
                    # the drain projections (pair B's on scalar/gpsimd,
                    # where sem-blocking is harmless).
                    c0 = ci * CHUNK
                    bounce_in = dram.tile([P, CHUNK], BF16,
                                          tag=f"binh{pair}",
                                          name=f"binh{ci}_{pair}", bufs=2)
                    bounce_out = dram.tile([grp * P, CHUNK], BF16,
                                           tag=f"bouth{pair}",
                                           name=f"bouth{ci}_{pair}", bufs=2)
                    (nc.sync if pair == 0 else nc.scalar).dma_start(
                        bounce_in[:], ot_sb[pair][:, c0:c0 + CHUNK])
                    nc.gpsimd.collective_compute(
                        "AllGather", mybir.AluOpType.bypass,
                        replica_groups=groups,
                        ins=[bounce_in.opt()], outs=[bounce_out.opt()])
                    tiles = {}
                    loads = []
                    lq = [nc.gpsimd, nc.scalar]
                    for r in range(grp):
                        k = 2 * r + pair
                        t = sb.tile([P, CHUNK], BF16, tag=f"ag{k}",
                                    name=f"ag{ci}_{k}", bufs=3)
                        loads.append((lq[r % 2], t[:],
                                      bounce_out[r * P:(r + 1) * P, :]))
                        tiles[k] = t
                    return tiles, loads

                def flush(jt_end=jt_end, ot_ps=ot_ps, pend=pend):
                    if pend[0] is None:
                        return
                    hp, jt, es, rel = pend[0]
                    pend[0] = None
                    for h2 in range(2):
                        h = 2 * hp + h2
                        nc.tensor.matmul(
                            ot_ps[hp][h2][:, rel:CHUNK],
                            lhsT=v_sb[jt][:, h * DHA:(h + 1) * DHA],
                            rhs=es[:, h2 * CHUNK + rel:(h2 + 1) * CHUNK],
                            start=(jt == 0), stop=(jt == jt_end - 1))
                    if jt == jt_end - 1:
                        do_pass_end(hp)

                for hp in range(2):
                    for jt in range(jt_end):
                        pos = ci + (hp * jt_end + jt) / (2 * jt_end)
                        drain_until(pos)
                        if jt == 0:
                            ot_ps[hp] = [
                                ps.tile([DHA, CHUNK], F32, tag=f"ot{h2}",
                                        name=f"ot{ci}_{hp}_{h2}", bufs=1)
                                for h2 in range(2)]
                        rel = max(0, (jt - jpc * ci)) * P
                        diag = jt >= jpc * ci

                        s2 = ps.tile([P, 2 * CHUNK], F32, tag="s2",
                                     name=f"s{ci}_{hp}_{jt}", bufs=2)
                        es = sb.tile([P, 2 * CHUNK], BF16, tag="es",
                                     name=f"es{ci}_{hp}_{jt}", bufs=4)

                        for h2 in range(2):
                            # S^T tile = K_h @ Q_h^T (row-tiled, K=64; the
                            # two heads run concurrently in the PE array)
                            nc.tensor.matmul(
                                s2[:, h2 * CHUNK + rel:(h2 + 1) * CHUNK],
                                lhsT=kt_sb[hp][h2 * DH:(h2 + 1) * DH,
                                               jt * P:(jt + 1) * P],
                                rhs=qt_sb[hp][h2 * DH:(h2 + 1) * DH,
                                              c0 + rel:c0 + CHUNK],
                                start=True, stop=not diag,
                                tile_position=(h2 * DH, 0))
                        if diag:
                            # additive causal mask accumulated into the S
                            # psum: identity.T @ maskneg tile (PE-local, no
                            # DVE in the S->exp->PV chain)
                            for h2 in range(2):
                                nc.tensor.matmul(
                                    s2[:, h2 * CHUNK + rel:
                                       h2 * CHUNK + rel + P],
                                    lhsT=mask_sb[:, 2 * P:3 * P],
                                    rhs=mask_sb[:, h2 * P:(h2 + 1) * P],
                                    start=False, stop=True)
                        # one exp for both heads (both psum banks); flat AP
                        # off the diagonal (3D APs cost ~190ns extra on ACT)
                        if rel == 0:
                            ei = nc.scalar.activation(es[:], s2[:], ActFn.Exp)
                        else:
                            ei = nc.scalar.activation(
                                es.rearrange("p (t c) -> p t c",
                                             t=2)[:, :, rel:],
                                s2.rearrange("p (t c) -> p t c",
                                             t=2)[:, :, rel:],
                                ActFn.Exp)
                        last_exp[0] = ei.ins
                        # software pipeline: independent queued matmuls keep
                        # the PE streaming while exp(t) runs on ACT, then the
                        # previous slot's PV issues
                        step_work(2, pos)
                        flush()
                        pend[0] = (hp, jt, es, rel)
                flush()

                if last:
                    agt = dict(last_parts)
                    last_parts = {}
                else:
                    agt, agloads = emit_ag_full(ci, binf)
                slices = [(agt[k], 0) for k in range(nko)]
                evens = [k for k in range(nko) if k % 2 == 0]
                odds = [k for k in range(nko) if k % 2 == 1]
                nm = HS // P
                if not last:
                    # all projections run in the final drain: the serial CC
                    # stream only comes up ~60-85us in (runtime barrier), so
                    # no gather lands before the body is nearly done — the
                    # drain's ~27us of proj matmuls overlaps the CC tail
                    for m in range(nm):
                        work_late.append(
                            [INF, INF,
                             g_proj(ci, m, slices, list(range(nko)),
                                    loads=agloads if m == 0 else None)])
                else:
                    # split each output block's projection: the even k-tiles
                    # (from the pair-A AllGather) run while pair-B flies
                    op_box = {}
                    for m in range(nm):
                        work_late.append(
                            [INF, INF,
                             g_proj(ci, m, slices, evens, op_box=op_box,
                                    evac=False,
                                    loads=last_loads[0] if m == 0 else None)])
                    for m in range(nm):
                        work_late.append(
                            [INF, INF,
                             g_proj(ci, m, slices, odds, op_box=op_box,
                                    loads=last_loads[1] if m == 0 else None)])

            # final drain: ungated leftovers first, then the AG-gated
            # projections in order — everything but the last pair-B
            # projections is ready PE work spanning the last AG's flight
            while work:
                for _ in work[0][1]:
                    pass
                work.popleft()
            while work_late:
                for _ in work_late[0][2]:
                    pass
                work_late.popleft()

    if compile:
        nc.compile()
    return nc


def make_in_maps(x, Wq, Wk, Wv, Wo, n_cores=N_CORES):
    import ml_dtypes
    bf16 = ml_dtypes.bfloat16
    scale = np.float32(DH ** -0.5)
    # additive band mask for the diagonal j-tile of S^T [j,i]: 0 where
    # j <= i, -30000 where j > i (duplicated side by side for both heads),
    # plus the identity used to matmul-accumulate it into the S psum
    mask_b = np.where(np.triu(np.ones((P, P), np.float32)) > 0,
                      0.0, MASKNEG).astype(np.float32)
    ident = np.eye(P, dtype=np.float32)
    mask2 = np.concatenate([mask_b, mask_b, ident], axis=1).astype(bf16)

    def pack(sl):
        # [ntk*128, HS] -> [128, ntk*HS]: row p holds k-tile blocks side by
        # side so the whole weight is one contiguous-row DMA
        ntk = sl.shape[0] // P
        return np.ascontiguousarray(
            sl.reshape(ntk, P, HS).transpose(1, 0, 2).reshape(P, ntk * HS)
        ).astype(bf16)

    def pack_pm(sl):
        # pair-major: [nk*128, 2*128] -> [128, 2*nk*128] with
        # out[p, (pair*nk + k)*128 + j] = sl[k*128 + p, pair*128 + j],
        # so each head-pair's weights are one contiguous half
        ntk = sl.shape[0] // P
        return np.ascontiguousarray(
            sl.reshape(ntk, P, 2, P).transpose(1, 2, 0, 3).reshape(
                P, 2 * ntk * P)
        ).astype(bf16)

    in_maps = []
    for c in range(n_cores):
        b, r = divmod(c, 4)
        hs = r * HS
        in_maps.append({
            "xT": np.ascontiguousarray(x[b].T).astype(bf16),
            "wq": pack_pm(Wq[:, hs:hs + HS] * scale),
            "wk": pack_pm(Wk[:, hs:hs + HS]),
            "wv": pack(Wv[:, hs:hs + HS]),
            "wo": pack(Wo[:, hs:hs + HS]),
            "mask_c": mask2,
        })
    return in_maps


def assemble_out(results, B, seq, n_cores=N_CORES):
    out = np.empty((B, seq, INNER), np.float32)
    for c in range(n_cores):
        b, r = divmod(c, 4)
        out[b][:, r * HS:(r + 1) * HS] = results[c]["outT"].T.astype(
            np.float32)
    return out


_NC_CACHE = {}


def kernel(x, Wq, Wk, Wv, Wo):
    from concourse import bass_utils
    x = np.asarray(x, np.float32)
    B, seq, dim = x.shape
    key = (seq, dim)
    if key not in _NC_CACHE:
        _NC_CACHE[key] = build_nc(seq=seq, dim=dim)
    nc = _NC_CACHE[key]
    in_maps = make_in_maps(x, np.asarray(Wq, np.float32),
                           np.asarray(Wk, np.float32),
                           np.asarray(Wv, np.float32),
                           np.asarray(Wo, np.float32))
    res = bass_utils.run_bass_kernel_spmd(
        nc, in_maps, core_ids=list(range(N_CORES)))
    return assemble_out(res.results, B, seq)


# revision 31
# speedup vs baseline: 1.0170x; 1.0170x over previous
"""Distributed causal multi-head attention for one TRN2 chip (8 NeuronCores).

Sharding: batch (2) x head-groups (4 heads/core) -> 8 cores.
Core c handles batch c//4, heads [ (c%4)*4 , (c%4)*4+4 ).
Per core: QKV projections for its 4 heads, flash-style causal attention
with scores kept transposed (S^T = K @ Q^T) so the PV product needs no
transposes; V is augmented with a ones column so the softmax denominators
fall out of the same matmul (row 64 of each head's O^T psum).  Then an
AllGather of the attention output (pre-Wo, 4-core group = one batch) and
a column-sliced output projection.  Host assembles the 8 column/batch
shards.  Compute dtype bf16 (PSUM accumulation fp32), softmax in fp32.

Scheduling: the attention inner loop is software-pipelined one slot deep
(S^T of slot t+1 before PV of slot t) with a FINE-GRAINED work queue:
QKV-projection and output-projection matmuls are generators yielding one
matmul at a time, and two steps are popped between each slot's S and the
previous slot's PV.  The PE therefore always has ~1us of independent
work queued while the exp of the previous slot completes on ACT, which
keeps the in-order PE queue streaming (and the tensor engine at high
p-state) instead of stalling ~300-500ns per slot.  The causal mask is
applied as an identity-matmul accumulation of an additive -30000 tile
into the S psum (PE-local), removing the DVE multiply from the
S->exp->PV critical path.  Q/K/V psum evacuations ride on DVE; exp owns
ACT exclusively.  Work items carry (deadline, earliest) slot positions:
deadlines guarantee S/PV dependencies are emitted in time on the
in-order queues, earliest-gating keeps AllGather-dependent projections
from stalling the PE before their gather lands, and the last chunk's
projections are reserved for the final drain where they fill the last
AllGather's flight time.  Inputs arrive as one wide DMA per weight
(host-packed 4KB rows; wq/wk pair-major so the first matmul waits only
on a 256KB half) and the x k-tiles load in chunk-column waves over FOUR
DMA queues for chunk 0 (sync/gpsimd/scalar/vector) so the first QKV
matmul starts ~9us in.  Softmax normalization multiplies the O^T psum
directly with the gpsimd-broadcast reciprocal of the ones-column sums;
the final chunk's AllGather bounce/tile DMAs use otherwise-idle queues.
"""

import sys
from collections import deque

import numpy as np

sys.path.insert(0, "/opt/trn_rl_repo")

import concourse.bass as bass  # noqa: E402
import concourse.bacc as bacc  # noqa: E402
import concourse.tile as tile  # noqa: E402
import concourse.mybir as mybir  # noqa: E402
from concourse.tile_rust import add_dep_helper  # noqa: E402

F32 = mybir.dt.float32
BF16 = mybir.dt.bfloat16
ActFn = mybir.ActivationFunctionType

P = 128          # partition dim
CHUNK = 512      # i-chunk (matmul moving free dim, one psum bank of fp32)
DH = 64          # head dim
HPC = 4          # heads per core
HS = HPC * DH    # 256 per-core inner slice
DHA = DH + 1     # augmented head dim (ones column for softmax sums)
INNER = 1024     # total inner dim (16 heads x 64)
N_CORES = 8
GROUPS = [[0, 1, 2, 3], [4, 5, 6, 7]]
MASKNEG = -30000.0   # additive causal mask (exp(x-30000) == 0 in fp32)


def build_nc(seq=2048, dim=1024, n_cores=N_CORES, groups=GROUPS, compile=True):
    """Build the SPMD Bass graph (identical on all cores)."""
    nch = seq // CHUNK          # i-chunks
    jpc = CHUNK // P            # j-tiles per chunk (4)
    njt = seq // P              # j-tiles
    nk = dim // P               # feature k-tiles
    nko = INNER // P            # inner k-tiles for the output projection
    grp = len(groups[0])        # replica group size (4)

    nc = bacc.Bacc("TRN2", target_bir_lowering=False, debug=False,
                   enable_asserts=False, num_devices=n_cores)

    xT = nc.dram_tensor("xT", [dim, seq], BF16, kind="ExternalInput").ap()
    # weights host-packed so each SBUF partition row is one contiguous DMA
    # row: [p, k*HS+j] = W[k*128+p, j]
    wq = nc.dram_tensor("wq", [P, nk * HS], BF16, kind="ExternalInput").ap()
    wk = nc.dram_tensor("wk", [P, nk * HS], BF16, kind="ExternalInput").ap()
    wv = nc.dram_tensor("wv", [P, nk * HS], BF16, kind="ExternalInput").ap()
    wo = nc.dram_tensor("wo", [P, nko * HS], BF16, kind="ExternalInput").ap()
    # [P, 0:2P] = additive causal mask duplicated for both heads,
    # [P, 2P:3P] = identity (mask accumulates into the S psum via matmul)
    mask_c = nc.dram_tensor("mask_c", [P, 3 * P], BF16,
                            kind="ExternalInput").ap()
    outT = nc.dram_tensor("outT", [HS, seq], BF16, kind="ExternalOutput").ap()

    with tile.TileContext(nc) as tc:
        with tc.tile_pool(name="sb", bufs=1) as sb, \
             tc.tile_pool(name="ps", bufs=1, space="PSUM") as ps, \
             tc.tile_pool(name="dram", bufs=1, space="DRAM") as dram:

            # ---- load inputs ----
            xt = [sb.tile([P, seq], BF16, tag=f"xt{k}", name=f"xt{k}")
                  for k in range(nk)]
            wq_sb = sb.tile([P, nk * HS], BF16, tag="wq", name="wq")
            wk_sb = sb.tile([P, nk * HS], BF16, tag="wk", name="wk")
            wv_sb = sb.tile([P, nk * HS], BF16, tag="wv", name="wv")
            wo_sb = sb.tile([P, nko * HS], BF16, tag="wo", name="wo")
            mask_sb = sb.tile([P, 3 * P], BF16, tag="mask", name="mask")

            # chunk-0 x wave over the three DMA-capable queues so the first
            # QKV matmul can start early; later waves on sync/gpsimd only
            # (scalar carries the wq/wk second halves, then exp owns it).
            # No CC warmup op: the collective barrier init is enqueued by
            # the runtime at a fixed ~22us regardless, and a warmup AG only
            # steals ~8us of the serial CC stream.
            q3 = [nc.sync, nc.gpsimd, nc.scalar]
            qs = [nc.sync, nc.gpsimd]

            def ld_x(k, lo, hi, queue):
                if lo < hi:
                    queue.dma_start(xt[k][:, lo:hi],
                                    xT[k * P:(k + 1) * P, lo:hi])
            hw = nk * P
            # first transfers on each queue gate the first Q matmul
            nc.scalar.dma_start(wq_sb[:, 0:hw], wq[:, 0:hw])
            for k in range(nk):
                ld_x(k, 0, CHUNK, q3[k % 3])
            nc.scalar.dma_start(wk_sb[:, 0:hw], wk[:, 0:hw])
            nc.gpsimd.dma_start(wv_sb[:], wv[:])
            nc.gpsimd.dma_start(mask_sb[:], mask_c[:])
            nc.scalar.dma_start(wq_sb[:, hw:2 * hw], wq[:, hw:2 * hw])
            nc.scalar.dma_start(wk_sb[:, hw:2 * hw], wk[:, hw:2 * hw])
            for k in range(nk):
                ld_x(k, CHUNK, min(2 * CHUNK, seq), qs[k % 2])
            nc.gpsimd.dma_start(wo_sb[:], wo[:])
            for k in range(nk):
                ld_x(k, 2 * CHUNK, seq, qs[k % 2])

            def wsl(w, k, a, b):
                return w[:, k * HS + a:k * HS + b]

            def wsl_pm(w, pair, k):
                # pair-major packed wq/wk: [p, (pair*nk + k)*128 + j]
                return w[:, (pair * nk + k) * P:(pair * nk + k + 1) * P]

            # persistent QKV results
            qt_sb = [sb.tile([P, seq], BF16, tag=f"qt{p}", name=f"qt{p}")
                     for p in range(2)]
            kt_sb = [sb.tile([P, seq], BF16, tag=f"kt{p}", name=f"kt{p}")
                     for p in range(2)]
            v_sb = [sb.tile([P, HPC * DHA], BF16, tag=f"v{j}", name=f"v{j}")
                    for j in range(njt)]
            ot_sb = [sb.tile([P, seq], BF16, tag=f"ot{p}", name=f"ot{p}")
                     for p in range(2)]
            # ones columns of the augmented V are set once up front
            for jt in range(njt):
                nc.vector.memset(
                    v_sb[jt].rearrange("p (h d) -> p h d", h=HPC)[:, :,
                                                                  DH:DHA],
                    1.0)

            # ---- work generators: one ~512-row matmul per yield ----
            def g_kt(pair, ch):
                pt = ps.tile([P, CHUNK], F32, tag="misc",
                             name=f"ktps{pair}_{ch}", bufs=2)
                for k in range(nk):
                    nc.tensor.matmul(
                        pt[:], lhsT=wsl_pm(wk_sb, pair, k),
                        rhs=xt[k][:, ch * CHUNK:(ch + 1) * CHUNK],
                        start=(k == 0), stop=(k == nk - 1))
                    if k < nk - 1:
                        yield
                nc.vector.tensor_copy(
                    kt_sb[pair][:, ch * CHUNK:(ch + 1) * CHUNK], pt[:])
                yield

            def g_qt(pair, ch):
                pt = ps.tile([P, CHUNK], F32, tag="misc",
                             name=f"qps{pair}_{ch}", bufs=2)
                for k in range(nk):
                    nc.tensor.matmul(
                        pt[:], lhsT=wsl_pm(wq_sb, pair, k),
                        rhs=xt[k][:, ch * CHUNK:(ch + 1) * CHUNK],
                        start=(k == 0), stop=(k == nk - 1))
                    if k < nk - 1:
                        yield
                nc.vector.tensor_copy(
                    qt_sb[pair][:, ch * CHUNK:(ch + 1) * CHUNK], pt[:])
                yield

            def g_v(jt):
                # N=256 matmuls: two per yield to keep step granularity even
                pt = ps.tile([P, HS], F32, tag="misc",
                             name=f"vps{jt}", bufs=2)
                for k in range(nk):
                    nc.tensor.matmul(
                        pt[:], lhsT=xt[k][:, jt * P:(jt + 1) * P],
                        rhs=wsl(wv_sb, k, 0, HS),
                        start=(k == 0), stop=(k == nk - 1))
                    if k % 2 == 1 and k < nk - 1:
                        yield
                nc.vector.tensor_copy(
                    v_sb[jt].rearrange("p (h d) -> p h d", h=HPC)[:, :, 0:DH],
                    pt.rearrange("p (h d) -> p h d", h=HPC))
                yield

            def g_proj(ci, m, slices, korder, op_box=None, evac=True,
                       loads=None):
                # transposed output block: outT[m*128:(m+1)*128, chunk ci]
                # = Wo[:, m-slice].T @ attT[:, chunk] over the k-tiles in
                # `korder` (a partial pass parks its psum in op_box).
                # `loads` issues the gathered-tile DMAs as a prologue: the
                # generator only runs once its AllGather has landed, so the
                # DMAs don't sem-block their queues.
                c0 = ci * CHUNK
                if loads:
                    # pin the gathered-tile DMAs behind the last exp: the
                    # tile scheduler's collective cost model knows nothing
                    # of the ~55us CC-stream init, and without a real edge
                    # it places these sem-blocking DMAs mid-body on the
                    # scalar queue, stalling the exp stream for ~20us
                    for q, dst, src in loads:
                        di = q.dma_start(dst, src)
                        if last_exp[0] is not None:
                            add_dep_helper(di.ins, last_exp[0],
                                           reason="ag tile load gated to drain")
                        if last_cc[0] is not None:
                            # keep the final AllGather doorbell ahead of the
                            # drain's load clutter on the gpsimd queue
                            add_dep_helper(di.ins, last_cc[0], sync=False,
                                           reason="ag tile load after last doorbell")
                    yield
                first = op_box is None or op_box.get(m) is None
                if first:
                    op_ps = ps.tile([P, CHUNK], F32, tag="misc",
                                    name=f"op{ci}_{m}", bufs=2)
                else:
                    op_ps = op_box.pop(m)
                for n, k in enumerate(korder):
                    ag_t, coff = slices[k]
                    nc.tensor.matmul(
                        op_ps[:],
                        lhsT=wsl(wo_sb, k, m * P, (m + 1) * P),
                        rhs=ag_t[:, coff:coff + CHUNK],
                        start=(first and n == 0),
                        stop=(evac and n == len(korder) - 1))
                    if n < len(korder) - 1:
                        yield
                if not evac:
                    op_box[m] = op_ps
                    yield
                    return
                o_sb = sb.tile([P, CHUNK], BF16, tag="osb",
                               name=f"o{ci}_{m}", bufs=2)
                nc.vector.tensor_copy(o_sb[:], op_ps[:])
                nc.sync.dma_start(
                    outT[m * P:(m + 1) * P, c0:c0 + CHUNK], o_sb[:])
                yield

            # work:      [deadline, gen]  — ungated QKV work, FIFO
            # work_late: [earliest, deadline, gen] — AllGather-gated
            #            projections, FIFO; popped only once pos>=earliest
            work = deque()
            work_late = deque()
            INF = 1e9

            def drain_until(pos):
                # everything the upcoming S/PV slots depend on must already
                # be emitted on the in-order queues
                while work and work[0][0] <= pos:
                    for _ in work[0][1]:
                        pass
                    work.popleft()
                while work_late and work_late[0][1] <= pos:
                    for _ in work_late[0][2]:
                        pass
                    work_late.popleft()

            def step_work(n, pos, slack=0.0):
                # pop n steps; keep popping (up to 2 extra) while the head's
                # deadline is within `slack` — spreads what would otherwise
                # be a forced burst of matmuls at a pass boundary
                popped = 0
                while popped < n + 2:
                    if work:
                        if popped >= n and work[0][0] > pos + slack:
                            break
                        try:
                            next(work[0][1])
                            popped += 1
                        except StopIteration:
                            work.popleft()
                    elif work_late and work_late[0][0] <= pos:
                        if popped >= n:
                            break
                        try:
                            next(work_late[0][2])
                            popped += 1
                        except StopIteration:
                            work_late.popleft()
                    else:
                        break

            # ---- upfront: chunk-0 pass A deps emitted inline ----
            for _ in g_qt(0, 0):
                pass
            for _ in g_v(0):
                pass
            for _ in g_kt(0, 0):
                pass
            work.append([0.125, g_v(1)])
            work.append([0.25, g_v(2)])
            work.append([0.375, g_v(3)])
            work.append([0.5, g_qt(1, 0)])
            work.append([0.5, g_kt(1, 0)])

            # ---- attention chunks ----
            last_parts = {}
            last_loads = {}
            last_exp = [None]
            last_cc = [None]
            for ci in range(nch):
                jt_end = jpc * (ci + 1)
                c0 = ci * CHUNK
                last = ci == nch - 1

                if ci + 1 < nch:
                    njt_end = jpc * (ci + 2)
                    work.append([ci + 1, g_qt(0, ci + 1)])
                    work.append([ci + 1, g_kt(0, ci + 1)])
                    for jt in range(jpc * (ci + 1), jpc * (ci + 2)):
                        # consumed by PV(jt) in pass A of chunk ci+1
                        work.append(
                            [ci + 1 + jt / (2 * njt_end), g_v(jt)])
                    work.append([ci + 1.5, g_qt(1, ci + 1)])
                    work.append([ci + 1.5, g_kt(1, ci + 1)])

                # one bounce buffer per chunk: with bufs=2 a later chunk's
                # bounce DMA WAR-waits for an earlier AllGather to consume
                # its buffer, sem-blocking the sync queue for tens of us
                binf = None if last else dram.tile(
                    [2 * P, CHUNK], BF16, tag="binf", name=f"binf{ci}",
                    bufs=4)

                ot_ps = {}
                pend = [None]

                def do_pass_end(hp, ci=ci, c0=c0, last=last, binf=binf,
                                ot_ps=ot_ps):
                    # softmax normalize: rcp of each head's sum row (staged
                    # to SBUF), gpsimd broadcast, then one mul per head
                    # reading the O^T psum directly (no staging copy)
                    for h2 in range(2):
                        # reciprocal must read from SBUF: straight from the
                        # PSUM sum row it returns garbage on hardware (sim
                        # accepts it)
                        srow = sb.tile([1, CHUNK], F32, tag=f"sr{h2}",
                                       name=f"sr{ci}_{hp}_{h2}", bufs=2)
                        nc.vector.tensor_copy(srow[:],
                                              ot_ps[hp][h2][DH:DHA, :])
                        rcp = sb.tile([1, CHUNK], F32, tag=f"rcp{h2}",
                                      name=f"rcp{ci}_{hp}_{h2}", bufs=2)
                        nc.vector.reciprocal_approx_fast(rcp[:], srow[:])
                        bc_sb = sb.tile([DH, CHUNK], F32, tag=f"bc{h2}",
                                        name=f"bc{ci}_{hp}_{h2}", bufs=2)
                        nc.gpsimd.partition_broadcast(bc_sb[:], rcp[:],
                                                      channels=DH)
                        nc.vector.tensor_mul(
                            ot_sb[hp][h2 * DH:(h2 + 1) * DH, c0:c0 + CHUNK],
                            ot_ps[hp][h2][0:DH, :],
                            bc_sb[:])
                    if last:
                        tiles, loads = emit_ag_pair(ci, hp)
                        last_parts.update(tiles)
                        last_loads[hp] = loads
                    else:
                        nc.sync.dma_start(
                            binf[hp * P:(hp + 1) * P, :],
                            ot_sb[hp][:, c0:c0 + CHUNK])

                def emit_ag_full(ci, bounce_in):
                    # one AllGather for both head pairs of chunk ci (256KB;
                    # rank-major rows land so gathered row-block k*128 is
                    # exactly attT k-tile k).  Tiles are allocated here but
                    # DMA'd from the proj generator's prologue, after the
                    # gather has landed.  ch0/ch1 tiles ride sync (popped
                    # only once landed); ch2 is drain-only and its loads on
                    # gpsimd/scalar may block harmlessly there.
                    bounce_out = dram.tile([grp * 2 * P, CHUNK], BF16,
                                           tag="boutf", name=f"boutf{ci}",
                                           bufs=4)
                    nc.gpsimd.collective_compute(
                        "AllGather", mybir.AluOpType.bypass,
                        replica_groups=groups,
                        ins=[bounce_in.opt()], outs=[bounce_out.opt()])
                    tiles = {}
                    loads = []
                    for k in range(nko):
                        t = sb.tile([P, CHUNK], BF16, tag=f"ag{k}",
                                    name=f"ag{ci}_{k}", bufs=3)
                        q = nc.gpsimd if k % 2 == 0 else nc.scalar
                        loads.append(
                            (q, t[:], bounce_out[k * P:(k + 1) * P, :]))
                        tiles[k] = t
                    return tiles, loads

                def emit_ag_pair(ci, pair):
                    # half AllGather (one head pair) of the last chunk —
                    # fired right after that pair's normalize.  Gathered
                    # k-tiles land at k = 2r+pair.  Pair B's bounce rides
                    # the then-idle scalar queue; tile DMAs are issued from
                    # the drain projections (pair B's on scalar/gpsimd,
                    # where sem-blocking is harmless).
                    c0 = ci * CHUNK
                    bounce_in = dram.tile([P, CHUNK], BF16,
                                          tag=f"binh{pair}",
                                          name=f"binh{ci}_{pair}", bufs=2)
                    bounce_out = dram.tile([grp * P, CHUNK], BF16,
                                           tag=f"bouth{pair}",
                                           name=f"bouth{ci}_{pair}", bufs=2)
                    nc.sync.dma_start(
                        bounce_in[:], ot_sb[pair][:, c0:c0 + CHUNK])
                    cc = nc.gpsimd.collective_compute(
                        "AllGather", mybir.AluOpType.bypass,
                        replica_groups=groups,
                        ins=[bounce_in.opt()], outs=[bounce_out.opt()])
                    last_cc[0] = cc.ins
                    tiles = {}
                    loads = []
                    lq = [nc.gpsimd, nc.scalar]
                    for r in range(grp):
                        k = 2 * r + pair
                        t = sb.tile([P, CHUNK], BF16, tag=f"ag{k}",
                                    name=f"ag{ci}_{k}", bufs=3)
                        loads.append((lq[r % 2], t[:],
                                      bounce_out[r * P:(r + 1) * P, :]))
                        tiles[k] = t
                    return tiles, loads

                def flush(jt_end=jt_end, ot_ps=ot_ps, pend=pend):
                    if pend[0] is None:
                        return
                    hp, jt, es, rel = pend[0]
                    pend[0] = None
                    for h2 in range(2):
                        h = 2 * hp + h2
                        nc.tensor.matmul(
                            ot_ps[hp][h2][:, rel:CHUNK],
                            lhsT=v_sb[jt][:, h * DHA:(h + 1) * DHA],
                            rhs=es[:, h2 * CHUNK + rel:(h2 + 1) * CHUNK],
                            start=(jt == 0), stop=(jt == jt_end - 1))
                    if jt == jt_end - 1:
                        do_pass_end(hp)

                for hp in range(2):
                    for jt in range(jt_end):
                        pos = ci + (hp * jt_end + jt) / (2 * jt_end)
                        drain_until(pos)
                        if jt == 0:
                            ot_ps[hp] = [
                                ps.tile([DHA, CHUNK], F32, tag=f"ot{h2}",
                                        name=f"ot{ci}_{hp}_{h2}", bufs=1)
                                for h2 in range(2)]
                        rel = max(0, (jt - jpc * ci)) * P
                        diag = jt >= jpc * ci

                        s2 = ps.tile([P, 2 * CHUNK], F32, tag="s2",
                                     name=f"s{ci}_{hp}_{jt}", bufs=2)
                        es = sb.tile([P, 2 * CHUNK], BF16, tag="es",
                                     name=f"es{ci}_{hp}_{jt}", bufs=4)

                        for h2 in range(2):
                            # S^T tile = K_h @ Q_h^T (row-tiled, K=64; the
                            # two heads run concurrently in the PE array)
                            nc.tensor.matmul(
                                s2[:, h2 * CHUNK + rel:(h2 + 1) * CHUNK],
                                lhsT=kt_sb[hp][h2 * DH:(h2 + 1) * DH,
                                               jt * P:(jt + 1) * P],
                                rhs=qt_sb[hp][h2 * DH:(h2 + 1) * DH,
                                              c0 + rel:c0 + CHUNK],
                                start=True, stop=not diag,
                                tile_position=(h2 * DH, 0))
                        if diag:
                            # additive causal mask accumulated into the S
                            # psum: identity.T @ maskneg tile (PE-local, no
                            # DVE in the S->exp->PV chain)
                            for h2 in range(2):
                                nc.tensor.matmul(
                                    s2[:, h2 * CHUNK + rel:
                                       h2 * CHUNK + rel + P],
                                    lhsT=mask_sb[:, 2 * P:3 * P],
                                    rhs=mask_sb[:, h2 * P:(h2 + 1) * P],
                                    start=False, stop=True)
                        # one exp for both heads (both psum banks); flat AP
                        # off the diagonal (3D APs cost ~190ns extra on ACT)
                        if rel == 0:
                            ei = nc.scalar.activation(es[:], s2[:], ActFn.Exp)
                        else:
                            ei = nc.scalar.activation(
                                es.rearrange("p (t c) -> p t c",
                                             t=2)[:, :, rel:],
                                s2.rearrange("p (t c) -> p t c",
                                             t=2)[:, :, rel:],
                                ActFn.Exp)
                        last_exp[0] = ei.ins
                        # software pipeline: independent queued matmuls keep
                        # the PE streaming while exp(t) runs on ACT, then the
                        # previous slot's PV issues
                        step_work(2, pos, slack=2.5 / (2 * jt_end))
                        flush()
                        pend[0] = (hp, jt, es, rel)
                flush()

                if last:
                    agt = dict(last_parts)
                    last_parts = {}
                else:
                    agt, agloads = emit_ag_full(ci, binf)
                slices = [(agt[k], 0) for k in range(nko)]
                evens = [k for k in range(nko) if k % 2 == 0]
                odds = [k for k in range(nko) if k % 2 == 1]
                nm = HS // P
                if not last:
                    # all projections run in the final drain: the serial CC
                    # stream only comes up ~60-85us in (runtime barrier), so
                    # no gather lands before the body is nearly done — the
                    # drain's ~27us of proj matmuls overlaps the CC tail
                    for m in range(nm):
                        work_late.append(
                            [INF, INF,
                             g_proj(ci, m, slices, list(range(nko)),
                                    loads=agloads if m == 0 else None)])
                else:
                    # split each output block's projection: the even k-tiles
                    # (from the pair-A AllGather) run while pair-B flies
                    op_box = {}
                    for m in range(nm):
                        work_late.append(
                            [INF, INF,
                             g_proj(ci, m, slices, evens, op_box=op_box,
                                    evac=False,
                                    loads=last_loads[0] if m == 0 else None)])
                    for m in range(nm):
                        work_late.append(
                            [INF, INF,
                             g_proj(ci, m, slices, odds, op_box=op_box,
                                    loads=last_loads[1] if m == 0 else None)])

            # final drain: ungated leftovers first, then the AG-gated
            # projections in order — everything but the last pair-B
            # projections is ready PE work spanning the last AG's flight
            while work:
                for _ in work[0][1]:
                    pass
                work.popleft()
            while work_late:
                for _ in work_late[0][2]:
                    pass
                work_late.popleft()

    if compile:
        nc.compile()
    return nc


def make_in_maps(x, Wq, Wk, Wv, Wo, n_cores=N_CORES):
    import ml_dtypes
    bf16 = ml_dtypes.bfloat16
    scale = np.float32(DH ** -0.5)
    # additive band mask for the diagonal j-tile of S^T [j,i]: 0 where
    # j <= i, -30000 where j > i (duplicated side by side for both heads),
    # plus the identity used to matmul-accumulate it into the S psum
    mask_b = np.where(np.triu(np.ones((P, P), np.float32)) > 0,
                      0.0, MASKNEG).astype(np.float32)
    ident = np.eye(P, dtype=np.float32)
    mask2 = np.concatenate([mask_b, mask_b, ident], axis=1).astype(bf16)

    def pack(sl):
        # [ntk*128, HS] -> [128, ntk*HS]: row p holds k-tile blocks side by
        # side so the whole weight is one contiguous-row DMA
        ntk = sl.shape[0] // P
        return np.ascontiguousarray(
            sl.reshape(ntk, P, HS).transpose(1, 0, 2).reshape(P, ntk * HS)
        ).astype(bf16)

    def pack_pm(sl):
        # pair-major: [nk*128, 2*128] -> [128, 2*nk*128] with
        # out[p, (pair*nk + k)*128 + j] = sl[k*128 + p, pair*128 + j],
        # so each head-pair's weights are one contiguous half
        ntk = sl.shape[0] // P
        return np.ascontiguousarray(
            sl.reshape(ntk, P, 2, P).transpose(1, 2, 0, 3).reshape(
                P, 2 * ntk * P)
        ).astype(bf16)

    in_maps = []
    for c in range(n_cores):
        b, r = divmod(c, 4)
        hs = r * HS
        in_maps.append({
            "xT": np.ascontiguousarray(x[b].T).astype(bf16),
            "wq": pack_pm(Wq[:, hs:hs + HS] * scale),
            "wk": pack_pm(Wk[:, hs:hs + HS]),
            "wv": pack(Wv[:, hs:hs + HS]),
            "wo": pack(Wo[:, hs:hs + HS]),
            "mask_c": mask2,
        })
    return in_maps


def assemble_out(results, B, seq, n_cores=N_CORES):
    out = np.empty((B, seq, INNER), np.float32)
    for c in range(n_cores):
        b, r = divmod(c, 4)
        out[b][:, r * HS:(r + 1) * HS] = results[c]["outT"].T.astype(
            np.float32)
    return out


_NC_CACHE = {}


def kernel(x, Wq, Wk, Wv, Wo):
    from concourse import bass_utils
    x = np.asarray(x, np.float32)
    B, seq, dim = x.shape
    key = (seq, dim)
    if key not in _NC_CACHE:
        _NC_CACHE[key] = build_nc(seq=seq, dim=dim)
    nc = _NC_CACHE[key]
    in_maps = make_in_maps(x, np.asarray(Wq, np.float32),
                           np.asarray(Wk, np.float32),
                           np.asarray(Wv, np.float32),
                           np.asarray(Wo, np.float32))
    res = bass_utils.run_bass_kernel_spmd(
        nc, in_maps, core_ids=list(range(N_CORES)))
    return assemble_out(res.results, B, seq)


# revision 32
# speedup vs baseline: 1.1002x; 1.0818x over previous
"""Distributed causal multi-head attention for one TRN2 chip (8 NeuronCores).

Sharding: batch (2) x head-groups (4 heads/core) -> 8 cores.
Core c handles batch c//4, heads [ (c%4)*4 , (c%4)*4+4 ).
Per core: QKV projections for its 4 heads, flash-style causal attention
with scores kept transposed (S^T = K @ Q^T) so the PV product needs no
transposes; V is augmented with a ones column so the softmax denominators
fall out of the same matmul (row 64 of each head's O^T psum).  Then an
AllGather of the attention output (pre-Wo, 4-core group = one batch) and
a column-sliced output projection.  Host assembles the 8 column/batch
shards.  Compute dtype bf16 (PSUM accumulation fp32), softmax in fp32.

Scheduling: the attention inner loop is software-pipelined one slot deep
(S^T of slot t+1 before PV of slot t) with a FINE-GRAINED work queue:
QKV-projection and output-projection matmuls are generators yielding one
matmul at a time, and two steps are popped between each slot's S and the
previous slot's PV.  The PE therefore always has ~1us of independent
work queued while the exp of the previous slot completes on ACT, which
keeps the in-order PE queue streaming (and the tensor engine at high
p-state) instead of stalling ~300-500ns per slot.  The causal mask is
applied as an identity-matmul accumulation of an additive -30000 tile
into the S psum (PE-local), removing the DVE multiply from the
S->exp->PV critical path.  Q/K/V psum evacuations ride on DVE; exp owns
ACT exclusively.  Work items carry (deadline, earliest) slot positions:
deadlines guarantee S/PV dependencies are emitted in time on the
in-order queues, earliest-gating keeps AllGather-dependent projections
from stalling the PE before their gather lands, and the last chunk's
projections are reserved for the final drain where they fill the last
AllGather's flight time.  Inputs arrive as one wide DMA per weight
(host-packed 4KB rows; wq/wk pair-major so the first matmul waits only
on a 256KB half) and the x k-tiles load in chunk-column waves over FOUR
DMA queues for chunk 0 (sync/gpsimd/scalar/vector) so the first QKV
matmul starts ~9us in.  Softmax normalization multiplies the O^T psum
directly with the gpsimd-broadcast reciprocal of the ones-column sums;
the final chunk's AllGather bounce/tile DMAs use otherwise-idle queues.
"""

import sys
from collections import deque

import numpy as np

sys.path.insert(0, "/opt/trn_rl_repo")

import concourse.bass as bass  # noqa: E402
import concourse.bacc as bacc  # noqa: E402
import concourse.tile as tile  # noqa: E402
import concourse.mybir as mybir  # noqa: E402
from concourse.tile_rust import add_dep_helper  # noqa: E402

F32 = mybir.dt.float32
BF16 = mybir.dt.bfloat16
ActFn = mybir.ActivationFunctionType

P = 128          # partition dim
CHUNK = 512      # i-chunk (matmul moving free dim, one psum bank of fp32)
DH = 64          # head dim
HPC = 4          # heads per core
HS = HPC * DH    # 256 per-core inner slice
DHA = DH + 1     # augmented head dim (ones column for softmax sums)
INNER = 1024     # total inner dim (16 heads x 64)
N_CORES = 8
GROUPS = [[0, 1, 2, 3], [4, 5, 6, 7]]
MASKNEG = -30000.0   # additive causal mask (exp(x-30000) == 0 in fp32)


def build_nc(seq=2048, dim=1024, n_cores=N_CORES, groups=GROUPS, compile=True):
    """Build the SPMD Bass graph (identical on all cores)."""
    nch = seq // CHUNK          # i-chunks
    jpc = CHUNK // P            # j-tiles per chunk (4)
    njt = seq // P              # j-tiles
    nk = dim // P               # feature k-tiles
    nko = INNER // P            # inner k-tiles for the output projection
    grp = len(groups[0])        # replica group size (4)

    nc = bacc.Bacc("TRN2", target_bir_lowering=False, debug=False,
                   enable_asserts=False, num_devices=n_cores)

    xT = nc.dram_tensor("xT", [dim, seq], BF16, kind="ExternalInput").ap()
    # weights host-packed so each SBUF partition row is one contiguous DMA
    # row: [p, k*HS+j] = W[k*128+p, j]
    wq = nc.dram_tensor("wq", [P, nk * HS], BF16, kind="ExternalInput").ap()
    wk = nc.dram_tensor("wk", [P, nk * HS], BF16, kind="ExternalInput").ap()
    wv = nc.dram_tensor("wv", [P, nk * HS], BF16, kind="ExternalInput").ap()
    wo = nc.dram_tensor("wo", [P, nko * HS], BF16, kind="ExternalInput").ap()
    # [P, 0:2P] = additive causal mask duplicated for both heads,
    # [P, 2P:3P] = identity (mask accumulates into the S psum via matmul)
    mask_c = nc.dram_tensor("mask_c", [P, 3 * P], BF16,
                            kind="ExternalInput").ap()
    outT = nc.dram_tensor("outT", [HS, seq], BF16, kind="ExternalOutput").ap()

    with tile.TileContext(nc) as tc:
        with tc.tile_pool(name="sb", bufs=1) as sb, \
             tc.tile_pool(name="ps", bufs=1, space="PSUM") as ps, \
             tc.tile_pool(name="dram", bufs=1, space="DRAM") as dram:

            # ---- load inputs ----
            xt = [sb.tile([P, seq], BF16, tag=f"xt{k}", name=f"xt{k}")
                  for k in range(nk)]
            wq_sb = sb.tile([P, nk * HS], BF16, tag="wq", name="wq")
            wk_sb = sb.tile([P, nk * HS], BF16, tag="wk", name="wk")
            wv_sb = sb.tile([P, nk * HS], BF16, tag="wv", name="wv")
            wo_sb = sb.tile([P, nko * HS], BF16, tag="wo", name="wo")
            mask_sb = sb.tile([P, 3 * P], BF16, tag="mask", name="mask")

            # chunk-0 x wave over the three DMA-capable queues so the first
            # QKV matmul can start early; later waves on sync/gpsimd only
            # (scalar carries the wq/wk second halves, then exp owns it).
            # No CC warmup op: the collective barrier init is enqueued by
            # the runtime at a fixed ~22us regardless, and a warmup AG only
            # steals ~8us of the serial CC stream.
            q3 = [nc.sync, nc.gpsimd, nc.scalar]
            qs = [nc.sync, nc.gpsimd]

            def ld_x(k, lo, hi, queue):
                if lo < hi:
                    queue.dma_start(xt[k][:, lo:hi],
                                    xT[k * P:(k + 1) * P, lo:hi])
            hw = nk * P
            # first transfers on each queue gate the first Q matmul
            nc.scalar.dma_start(wq_sb[:, 0:hw], wq[:, 0:hw])
            for k in range(nk):
                ld_x(k, 0, CHUNK, q3[k % 3])
            nc.scalar.dma_start(wk_sb[:, 0:hw], wk[:, 0:hw])
            nc.gpsimd.dma_start(wv_sb[:], wv[:])
            nc.gpsimd.dma_start(mask_sb[:], mask_c[:])
            nc.scalar.dma_start(wq_sb[:, hw:2 * hw], wq[:, hw:2 * hw])
            nc.scalar.dma_start(wk_sb[:, hw:2 * hw], wk[:, hw:2 * hw])
            for k in range(nk):
                ld_x(k, CHUNK, min(2 * CHUNK, seq), qs[k % 2])
            nc.gpsimd.dma_start(wo_sb[:], wo[:])
            for k in range(nk):
                ld_x(k, 2 * CHUNK, seq, qs[k % 2])

            def wsl(w, k, a, b):
                return w[:, k * HS + a:k * HS + b]

            def wsl_pm(w, pair, k):
                # pair-major packed wq/wk: [p, (pair*nk + k)*128 + j]
                return w[:, (pair * nk + k) * P:(pair * nk + k + 1) * P]

            # persistent QKV results
            qt_sb = [sb.tile([P, seq], BF16, tag=f"qt{p}", name=f"qt{p}")
                     for p in range(2)]
            kt_sb = [sb.tile([P, seq], BF16, tag=f"kt{p}", name=f"kt{p}")
                     for p in range(2)]
            v_sb = [sb.tile([P, HPC * DHA], BF16, tag=f"v{j}", name=f"v{j}")
                    for j in range(njt)]
            ot_sb = [sb.tile([P, seq], BF16, tag=f"ot{p}", name=f"ot{p}")
                     for p in range(2)]
            # ones columns of the augmented V are set once up front
            for jt in range(njt):
                nc.vector.memset(
                    v_sb[jt].rearrange("p (h d) -> p h d", h=HPC)[:, :,
                                                                  DH:DHA],
                    1.0)

            # ---- work generators: one ~512-row matmul per yield ----
            def g_kt(pair, ch):
                pt = ps.tile([P, CHUNK], F32, tag="misc",
                             name=f"ktps{pair}_{ch}", bufs=2)
                for k in range(nk):
                    nc.tensor.matmul(
                        pt[:], lhsT=wsl_pm(wk_sb, pair, k),
                        rhs=xt[k][:, ch * CHUNK:(ch + 1) * CHUNK],
                        start=(k == 0), stop=(k == nk - 1))
                    if k < nk - 1:
                        yield
                nc.vector.tensor_copy(
                    kt_sb[pair][:, ch * CHUNK:(ch + 1) * CHUNK], pt[:])
                yield

            def g_qt(pair, ch):
                pt = ps.tile([P, CHUNK], F32, tag="misc",
                             name=f"qps{pair}_{ch}", bufs=2)
                for k in range(nk):
                    nc.tensor.matmul(
                        pt[:], lhsT=wsl_pm(wq_sb, pair, k),
                        rhs=xt[k][:, ch * CHUNK:(ch + 1) * CHUNK],
                        start=(k == 0), stop=(k == nk - 1))
                    if k < nk - 1:
                        yield
                nc.vector.tensor_copy(
                    qt_sb[pair][:, ch * CHUNK:(ch + 1) * CHUNK], pt[:])
                yield

            def g_v(jt):
                # N=256 matmuls: two per yield to keep step granularity even
                pt = ps.tile([P, HS], F32, tag="misc",
                             name=f"vps{jt}", bufs=2)
                for k in range(nk):
                    nc.tensor.matmul(
                        pt[:], lhsT=xt[k][:, jt * P:(jt + 1) * P],
                        rhs=wsl(wv_sb, k, 0, HS),
                        start=(k == 0), stop=(k == nk - 1))
                    if k % 2 == 1 and k < nk - 1:
                        yield
                nc.vector.tensor_copy(
                    v_sb[jt].rearrange("p (h d) -> p h d", h=HPC)[:, :, 0:DH],
                    pt.rearrange("p (h d) -> p h d", h=HPC))
                yield

            def g_proj(ci, m, slices, korder, op_box=None, evac=True,
                       loads=None):
                # transposed output block: outT[m*128:(m+1)*128, chunk ci]
                # = Wo[:, m-slice].T @ attT[:, chunk] over the k-tiles in
                # `korder` (a partial pass parks its psum in op_box).
                # `loads` issues the gathered-tile DMAs as a prologue: the
                # generator only runs once its AllGather has landed, so the
                # DMAs don't sem-block their queues.
                c0 = ci * CHUNK
                if loads:
                    # pin the gathered-tile DMAs behind the last exp: the
                    # tile scheduler's collective cost model knows nothing
                    # of the ~55us CC-stream init, and without a real edge
                    # it places these sem-blocking DMAs mid-body on the
                    # scalar queue, stalling the exp stream for ~20us
                    for q, dst, src in loads:
                        di = q.dma_start(dst, src)
                        if last_exp[0] is not None:
                            add_dep_helper(di.ins, last_exp[0],
                                           reason="ag tile load gated to drain")
                    yield
                first = op_box is None or op_box.get(m) is None
                if first:
                    op_ps = ps.tile([P, CHUNK], F32, tag="misc",
                                    name=f"op{ci}_{m}", bufs=2)
                else:
                    op_ps = op_box.pop(m)
                for n, k in enumerate(korder):
                    ag_t, coff = slices[k]
                    nc.tensor.matmul(
                        op_ps[:],
                        lhsT=wsl(wo_sb, k, m * P, (m + 1) * P),
                        rhs=ag_t[:, coff:coff + CHUNK],
                        start=(first and n == 0),
                        stop=(evac and n == len(korder) - 1))
                    if n < len(korder) - 1:
                        yield
                if not evac:
                    op_box[m] = op_ps
                    yield
                    return
                o_sb = sb.tile([P, CHUNK], BF16, tag="osb",
                               name=f"o{ci}_{m}", bufs=2)
                nc.vector.tensor_copy(o_sb[:], op_ps[:])
                nc.sync.dma_start(
                    outT[m * P:(m + 1) * P, c0:c0 + CHUNK], o_sb[:])
                yield

            # work:      [deadline, gen]  — ungated QKV work, FIFO
            # work_late: [earliest, deadline, gen] — AllGather-gated
            #            projections, FIFO; popped only once pos>=earliest
            work = deque()
            work_late = deque()
            INF = 1e9

            def drain_until(pos):
                # everything the upcoming S/PV slots depend on must already
                # be emitted on the in-order queues
                while work and work[0][0] <= pos:
                    for _ in work[0][1]:
                        pass
                    work.popleft()
                while work_late and work_late[0][1] <= pos:
                    for _ in work_late[0][2]:
                        pass
                    work_late.popleft()

            def step_work(n, pos, slack=0.0):
                # pop n steps; keep popping (up to 2 extra) while the head's
                # deadline is within `slack` — spreads what would otherwise
                # be a forced burst of matmuls at a pass boundary
                popped = 0
                while popped < n + 2:
                    if work:
                        if popped >= n and work[0][0] > pos + slack:
                            break
                        try:
                            next(work[0][1])
                            popped += 1
                        except StopIteration:
                            work.popleft()
                    elif work_late and work_late[0][0] <= pos:
                        if popped >= n:
                            break
                        try:
                            next(work_late[0][2])
                            popped += 1
                        except StopIteration:
                            work_late.popleft()
                    else:
                        break

            # ---- upfront: chunk-0 pass A deps emitted inline ----
            for _ in g_qt(0, 0):
                pass
            for _ in g_v(0):
                pass
            for _ in g_kt(0, 0):
                pass
            work.append([0.125, g_v(1)])
            work.append([0.25, g_v(2)])
            work.append([0.375, g_v(3)])
            work.append([0.5, g_qt(1, 0)])
            work.append([0.5, g_kt(1, 0)])

            # ---- attention chunks ----
            last_parts = {}
            last_loads = {}
            last_exp = [None]
            last_cc = [None]
            for ci in range(nch):
                jt_end = jpc * (ci + 1)
                c0 = ci * CHUNK
                last = ci == nch - 1

                if ci + 1 < nch:
                    njt_end = jpc * (ci + 2)
                    work.append([ci + 1, g_qt(0, ci + 1)])
                    work.append([ci + 1, g_kt(0, ci + 1)])
                    for jt in range(jpc * (ci + 1), jpc * (ci + 2)):
                        # consumed by PV(jt) in pass A of chunk ci+1
                        work.append(
                            [ci + 1 + jt / (2 * njt_end), g_v(jt)])
                    work.append([ci + 1.5, g_qt(1, ci + 1)])
                    work.append([ci + 1.5, g_kt(1, ci + 1)])

                # one bounce buffer per chunk: with bufs=2 a later chunk's
                # bounce DMA WAR-waits for an earlier AllGather to consume
                # its buffer, sem-blocking the sync queue for tens of us
                binf = None if last else dram.tile(
                    [2 * P, CHUNK], BF16, tag="binf", name=f"binf{ci}",
                    bufs=4)

                ot_ps = {}
                pend = [None]

                def do_pass_end(hp, ci=ci, c0=c0, last=last, binf=binf,
                                ot_ps=ot_ps):
                    # softmax normalize: rcp of each head's sum row (staged
                    # to SBUF), gpsimd broadcast, then one mul per head
                    # reading the O^T psum directly (no staging copy)
                    for h2 in range(2):
                        # reciprocal must read from SBUF: straight from the
                        # PSUM sum row it returns garbage on hardware (sim
                        # accepts it)
                        srow = sb.tile([1, CHUNK], F32, tag=f"sr{h2}",
                                       name=f"sr{ci}_{hp}_{h2}", bufs=2)
                        nc.vector.tensor_copy(srow[:],
                                              ot_ps[hp][h2][DH:DHA, :])
                        rcp = sb.tile([1, CHUNK], F32, tag=f"rcp{h2}",
                                      name=f"rcp{ci}_{hp}_{h2}", bufs=2)
                        nc.vector.reciprocal_approx_fast(rcp[:], srow[:])
                        bc_sb = sb.tile([DH, CHUNK], F32, tag=f"bc{h2}",
                                        name=f"bc{ci}_{hp}_{h2}", bufs=2)
                        nc.gpsimd.partition_broadcast(bc_sb[:], rcp[:],
                                                      channels=DH)
                        nc.vector.tensor_mul(
                            ot_sb[hp][h2 * DH:(h2 + 1) * DH, c0:c0 + CHUNK],
                            ot_ps[hp][h2][0:DH, :],
                            bc_sb[:])
                    if last:
                        tiles, loads = emit_ag_pair(ci, hp)
                        last_parts.update(tiles)
                        last_loads[hp] = loads
                    else:
                        nc.sync.dma_start(
                            binf[hp * P:(hp + 1) * P, :],
                            ot_sb[hp][:, c0:c0 + CHUNK])

                def emit_ag_full(ci, bounce_in):
                    # one AllGather for both head pairs of chunk ci (256KB;
                    # rank-major rows land so gathered row-block k*128 is
                    # exactly attT k-tile k).  Tiles are allocated here but
                    # DMA'd from the proj generator's prologue, after the
                    # gather has landed.  ch0/ch1 tiles ride sync (popped
                    # only once landed); ch2 is drain-only and its loads on
                    # gpsimd/scalar may block harmlessly there.
                    bounce_out = dram.tile([grp * 2 * P, CHUNK], BF16,
                                           tag="boutf", name=f"boutf{ci}",
                                           bufs=4)
                    nc.gpsimd.collective_compute(
                        "AllGather", mybir.AluOpType.bypass,
                        replica_groups=groups,
                        ins=[bounce_in.opt()], outs=[bounce_out.opt()])
                    tiles = {}
                    loads = []
                    for k in range(nko):
                        t = sb.tile([P, CHUNK], BF16, tag=f"ag{k}",
                                    name=f"ag{ci}_{k}", bufs=3)
                        loads.append(
                            (nc.scalar, t[:],
                             bounce_out[k * P:(k + 1) * P, :]))
                        tiles[k] = t
                    return tiles, loads

                def emit_ag_pair(ci, pair):
                    # half AllGather (one head pair) of the last chunk —
                    # fired right after that pair's normalize.  Gathered
                    # k-tiles land at k = 2r+pair.  Pair B's bounce rides
                    # the then-idle scalar queue; tile DMAs are issued from
                    # the drain projections (pair B's on scalar/gpsimd,
                    # where sem-blocking is harmless).
                    c0 = ci * CHUNK
                    bounce_in = dram.tile([P, CHUNK], BF16,
                                          tag=f"binh{pair}",
                                          name=f"binh{ci}_{pair}", bufs=2)
                    bounce_out = dram.tile([grp * P, CHUNK], BF16,
                                           tag=f"bouth{pair}",
                                           name=f"bouth{ci}_{pair}", bufs=2)
                    nc.sync.dma_start(
                        bounce_in[:], ot_sb[pair][:, c0:c0 + CHUNK])
                    cc = nc.gpsimd.collective_compute(
                        "AllGather", mybir.AluOpType.bypass,
                        replica_groups=groups,
                        ins=[bounce_in.opt()], outs=[bounce_out.opt()])
                    last_cc[0] = cc.ins
                    tiles = {}
                    loads = []
                    lq = [nc.sync, nc.scalar]
                    for r in range(grp):
                        k = 2 * r + pair
                        t = sb.tile([P, CHUNK], BF16, tag=f"ag{k}",
                                    name=f"ag{ci}_{k}", bufs=3)
                        loads.append((lq[r % 2], t[:],
                                      bounce_out[r * P:(r + 1) * P, :]))
                        tiles[k] = t
                    return tiles, loads

                def flush(jt_end=jt_end, ot_ps=ot_ps, pend=pend):
                    if pend[0] is None:
                        return
                    hp, jt, es, rel = pend[0]
                    pend[0] = None
                    for h2 in range(2):
                        h = 2 * hp + h2
                        nc.tensor.matmul(
                            ot_ps[hp][h2][:, rel:CHUNK],
                            lhsT=v_sb[jt][:, h * DHA:(h + 1) * DHA],
                            rhs=es[:, h2 * CHUNK + rel:(h2 + 1) * CHUNK],
                            start=(jt == 0), stop=(jt == jt_end - 1))
                    if jt == jt_end - 1:
                        do_pass_end(hp)

                for hp in range(2):
                    for jt in range(jt_end):
                        pos = ci + (hp * jt_end + jt) / (2 * jt_end)
                        drain_until(pos)
                        if jt == 0:
                            ot_ps[hp] = [
                                ps.tile([DHA, CHUNK], F32, tag=f"ot{h2}",
                                        name=f"ot{ci}_{hp}_{h2}", bufs=1)
                                for h2 in range(2)]
                        rel = max(0, (jt - jpc * ci)) * P
                        diag = jt >= jpc * ci

                        s2 = ps.tile([P, 2 * CHUNK], F32, tag="s2",
                                     name=f"s{ci}_{hp}_{jt}", bufs=2)
                        es = sb.tile([P, 2 * CHUNK], BF16, tag="es",
                                     name=f"es{ci}_{hp}_{jt}", bufs=8)

                        for h2 in range(2):
                            # S^T tile = K_h @ Q_h^T (row-tiled, K=64; the
                            # two heads run concurrently in the PE array)
                            nc.tensor.matmul(
                                s2[:, h2 * CHUNK + rel:(h2 + 1) * CHUNK],
                                lhsT=kt_sb[hp][h2 * DH:(h2 + 1) * DH,
                                               jt * P:(jt + 1) * P],
                                rhs=qt_sb[hp][h2 * DH:(h2 + 1) * DH,
                                              c0 + rel:c0 + CHUNK],
                                start=True, stop=not diag,
                                tile_position=(h2 * DH, 0))
                        if diag:
                            # additive causal mask accumulated into the S
                            # psum: identity.T @ maskneg tile (PE-local, no
                            # DVE in the S->exp->PV chain)
                            for h2 in range(2):
                                nc.tensor.matmul(
                                    s2[:, h2 * CHUNK + rel:
                                       h2 * CHUNK + rel + P],
                                    lhsT=mask_sb[:, 2 * P:3 * P],
                                    rhs=mask_sb[:, h2 * P:(h2 + 1) * P],
                                    start=False, stop=True)
                        # one exp for both heads (both psum banks); flat AP
                        # off the diagonal (3D APs cost ~190ns extra on ACT)
                        if rel == 0:
                            ei = nc.scalar.activation(es[:], s2[:], ActFn.Exp)
                        else:
                            ei = nc.scalar.activation(
                                es.rearrange("p (t c) -> p t c",
                                             t=2)[:, :, rel:],
                                s2.rearrange("p (t c) -> p t c",
                                             t=2)[:, :, rel:],
                                ActFn.Exp)
                        last_exp[0] = ei.ins
                        # software pipeline: independent queued matmuls keep
                        # the PE streaming while exp(t) runs on ACT, then the
                        # previous slot's PV issues
                        step_work(2, pos, slack=2.5 / (2 * jt_end))
                        flush()
                        pend[0] = (hp, jt, es, rel)
                flush()

                if last:
                    agt = dict(last_parts)
                    last_parts = {}
                else:
                    agt, agloads = emit_ag_full(ci, binf)
                slices = [(agt[k], 0) for k in range(nko)]
                evens = [k for k in range(nko) if k % 2 == 0]
                odds = [k for k in range(nko) if k % 2 == 1]
                nm = HS // P
                if not last:
                    # all projections run in the final drain: the serial CC
                    # stream only comes up ~60-85us in (runtime barrier), so
                    # no gather lands before the body is nearly done — the
                    # drain's ~27us of proj matmuls overlaps the CC tail
                    for m in range(nm):
                        work_late.append(
                            [INF, INF,
                             g_proj(ci, m, slices, list(range(nko)),
                                    loads=agloads if m == 0 else None)])
                else:
                    # split each output block's projection: the even k-tiles
                    # (from the pair-A AllGather) run while pair-B flies
                    op_box = {}
                    for m in range(nm):
                        work_late.append(
                            [INF, INF,
                             g_proj(ci, m, slices, evens, op_box=op_box,
                                    evac=False,
                                    loads=last_loads[0] if m == 0 else None)])
                    for m in range(nm):
                        work_late.append(
                            [INF, INF,
                             g_proj(ci, m, slices, odds, op_box=op_box,
                                    loads=last_loads[1] if m == 0 else None)])

            # final drain: ungated leftovers first, then the AG-gated
            # projections in order — everything but the last pair-B
            # projections is ready PE work spanning the last AG's flight
            while work:
                for _ in work[0][1]:
                    pass
                work.popleft()
            while work_late:
                for _ in work_late[0][2]:
                    pass
                work_late.popleft()

    if compile:
        nc.compile()
    return nc


def make_in_maps(x, Wq, Wk, Wv, Wo, n_cores=N_CORES):
    import ml_dtypes
    bf16 = ml_dtypes.bfloat16
    scale = np.float32(DH ** -0.5)
    # additive band mask for the diagonal j-tile of S^T [j,i]: 0 where
    # j <= i, -30000 where j > i (duplicated side by side for both heads),
    # plus the identity used to matmul-accumulate it into the S psum
    mask_b = np.where(np.triu(np.ones((P, P), np.float32)) > 0,
                      0.0, MASKNEG).astype(np.float32)
    ident = np.eye(P, dtype=np.float32)
    mask2 = np.concatenate([mask_b, mask_b, ident], axis=1).astype(bf16)

    def pack(sl):
        # [ntk*128, HS] -> [128, ntk*HS]: row p holds k-tile blocks side by
        # side so the whole weight is one contiguous-row DMA
        ntk = sl.shape[0] // P
        return np.ascontiguousarray(
            sl.reshape(ntk, P, HS).transpose(1, 0, 2).reshape(P, ntk * HS)
        ).astype(bf16)

    def pack_pm(sl):
        # pair-major: [nk*128, 2*128] -> [128, 2*nk*128] with
        # out[p, (pair*nk + k)*128 + j] = sl[k*128 + p, pair*128 + j],
        # so each head-pair's weights are one contiguous half
        ntk = sl.shape[0] // P
        return np.ascontiguousarray(
            sl.reshape(ntk, P, 2, P).transpose(1, 2, 0, 3).reshape(
                P, 2 * ntk * P)
        ).astype(bf16)

    in_maps = []
    for c in range(n_cores):
        b, r = divmod(c, 4)
        hs = r * HS
        in_maps.append({
            "xT": np.ascontiguousarray(x[b].T).astype(bf16),
            "wq": pack_pm(Wq[:, hs:hs + HS] * scale),
            "wk": pack_pm(Wk[:, hs:hs + HS]),
            "wv": pack(Wv[:, hs:hs + HS]),
            "wo": pack(Wo[:, hs:hs + HS]),
            "mask_c": mask2,
        })
    return in_maps


def assemble_out(results, B, seq, n_cores=N_CORES):
    out = np.empty((B, seq, INNER), np.float32)
    for c in range(n_cores):
        b, r = divmod(c, 4)
        out[b][:, r * HS:(r + 1) * HS] = results[c]["outT"].T.astype(
            np.float32)
    return out


_NC_CACHE = {}


def kernel(x, Wq, Wk, Wv, Wo):
    from concourse import bass_utils
    x = np.asarray(x, np.float32)
    B, seq, dim = x.shape
    key = (seq, dim)
    if key not in _NC_CACHE:
        _NC_CACHE[key] = build_nc(seq=seq, dim=dim)
    nc = _NC_CACHE[key]
    in_maps = make_in_maps(x, np.asarray(Wq, np.float32),
                           np.asarray(Wk, np.float32),
                           np.asarray(Wv, np.float32),
                           np.asarray(Wo, np.float32))
    res = bass_utils.run_bass_kernel_spmd(
        nc, in_maps, core_ids=list(range(N_CORES)))
    return assemble_out(res.results, B, seq)


# revision 33
# speedup vs baseline: 1.1083x; 1.0074x over previous
"""Distributed causal multi-head attention for one TRN2 chip (8 NeuronCores).

Sharding: batch (2) x head-groups (4 heads/core) -> 8 cores.
Core c handles batch c//4, heads [ (c%4)*4 , (c%4)*4+4 ).
Per core: QKV projections for its 4 heads, flash-style causal attention
with scores kept transposed (S^T = K @ Q^T) so the PV product needs no
transposes; V is augmented with a ones column so the softmax denominators
fall out of the same matmul (row 64 of each head's O^T psum).  Then an
AllGather of the attention output (pre-Wo, 4-core group = one batch) and
a column-sliced output projection.  Host assembles the 8 column/batch
shards.  Compute dtype bf16 (PSUM accumulation fp32), softmax in fp32.

Scheduling: the attention inner loop is software-pipelined one slot deep
(S^T of slot t+1 before PV of slot t) with a FINE-GRAINED work queue:
QKV-projection and output-projection matmuls are generators yielding one
matmul at a time, and two steps are popped between each slot's S and the
previous slot's PV.  The PE therefore always has ~1us of independent
work queued while the exp of the previous slot completes on ACT, which
keeps the in-order PE queue streaming (and the tensor engine at high
p-state) instead of stalling ~300-500ns per slot.  The causal mask is
applied as an identity-matmul accumulation of an additive -30000 tile
into the S psum (PE-local), removing the DVE multiply from the
S->exp->PV critical path.  Q/K/V psum evacuations ride on DVE; exp owns
ACT exclusively.  Work items carry (deadline, earliest) slot positions:
deadlines guarantee S/PV dependencies are emitted in time on the
in-order queues, earliest-gating keeps AllGather-dependent projections
from stalling the PE before their gather lands, and the last chunk's
projections are reserved for the final drain where they fill the last
AllGather's flight time.  Inputs arrive as one wide DMA per weight
(host-packed 4KB rows; wq/wk pair-major so the first matmul waits only
on a 256KB half) and the x k-tiles load in chunk-column waves over FOUR
DMA queues for chunk 0 (sync/gpsimd/scalar/vector) so the first QKV
matmul starts ~9us in.  Softmax normalization multiplies the O^T psum
directly with the gpsimd-broadcast reciprocal of the ones-column sums;
the final chunk's AllGather bounce/tile DMAs use otherwise-idle queues.
"""

import sys
from collections import deque

import numpy as np

sys.path.insert(0, "/opt/trn_rl_repo")

import concourse.bass as bass  # noqa: E402
import concourse.bacc as bacc  # noqa: E402
import concourse.tile as tile  # noqa: E402
import concourse.mybir as mybir  # noqa: E402
from concourse.tile_rust import add_dep_helper  # noqa: E402

F32 = mybir.dt.float32
BF16 = mybir.dt.bfloat16
ActFn = mybir.ActivationFunctionType

P = 128          # partition dim
CHUNK = 512      # i-chunk (matmul moving free dim, one psum bank of fp32)
DH = 64          # head dim
HPC = 4          # heads per core
HS = HPC * DH    # 256 per-core inner slice
DHA = DH + 1     # augmented head dim (ones column for softmax sums)
INNER = 1024     # total inner dim (16 heads x 64)
N_CORES = 8
GROUPS = [[0, 1, 2, 3], [4, 5, 6, 7]]
MASKNEG = -30000.0   # additive causal mask (exp(x-30000) == 0 in fp32)


def build_nc(seq=2048, dim=1024, n_cores=N_CORES, groups=GROUPS, compile=True):
    """Build the SPMD Bass graph (identical on all cores)."""
    nch = seq // CHUNK          # i-chunks
    jpc = CHUNK // P            # j-tiles per chunk (4)
    njt = seq // P              # j-tiles
    nk = dim // P               # feature k-tiles
    nko = INNER // P            # inner k-tiles for the output projection
    grp = len(groups[0])        # replica group size (4)

    nc = bacc.Bacc("TRN2", target_bir_lowering=False, debug=False,
                   enable_asserts=False, num_devices=n_cores)

    xT = nc.dram_tensor("xT", [dim, seq], BF16, kind="ExternalInput").ap()
    # weights host-packed so each SBUF partition row is one contiguous DMA
    # row: [p, k*HS+j] = W[k*128+p, j]
    wq = nc.dram_tensor("wq", [P, nk * HS], BF16, kind="ExternalInput").ap()
    wk = nc.dram_tensor("wk", [P, nk * HS], BF16, kind="ExternalInput").ap()
    wv = nc.dram_tensor("wv", [P, nk * HS], BF16, kind="ExternalInput").ap()
    wo = nc.dram_tensor("wo", [P, nko * HS], BF16, kind="ExternalInput").ap()
    # [P, 0:2P] = additive causal mask duplicated for both heads,
    # [P, 2P:3P] = identity (mask accumulates into the S psum via matmul)
    mask_c = nc.dram_tensor("mask_c", [P, 3 * P], BF16,
                            kind="ExternalInput").ap()
    outT = nc.dram_tensor("outT", [HS, seq], BF16, kind="ExternalOutput").ap()

    with tile.TileContext(nc) as tc:
        with tc.tile_pool(name="sb", bufs=1) as sb, \
             tc.tile_pool(name="ps", bufs=1, space="PSUM") as ps, \
             tc.tile_pool(name="dram", bufs=1, space="DRAM") as dram:

            # ---- load inputs ----
            xt = [sb.tile([P, seq], BF16, tag=f"xt{k}", name=f"xt{k}")
                  for k in range(nk)]
            wq_sb = sb.tile([P, nk * HS], BF16, tag="wq", name="wq")
            wk_sb = sb.tile([P, nk * HS], BF16, tag="wk", name="wk")
            wv_sb = sb.tile([P, nk * HS], BF16, tag="wv", name="wv")
            wo_sb = sb.tile([P, nko * HS], BF16, tag="wo", name="wo")
            mask_sb = sb.tile([P, 3 * P], BF16, tag="mask", name="mask")

            # chunk-0 x wave over the three DMA-capable queues so the first
            # QKV matmul can start early; later waves on sync/gpsimd only
            # (scalar carries the wq/wk second halves, then exp owns it).
            # No CC warmup op: the collective barrier init is enqueued by
            # the runtime at a fixed ~22us regardless, and a warmup AG only
            # steals ~8us of the serial CC stream.
            q3 = [nc.sync, nc.gpsimd, nc.scalar]
            qs = [nc.sync, nc.gpsimd]

            def ld_x(k, lo, hi, queue):
                if lo < hi:
                    queue.dma_start(xt[k][:, lo:hi],
                                    xT[k * P:(k + 1) * P, lo:hi])
            hw = nk * P
            # first transfers on each queue gate the first Q matmul
            nc.scalar.dma_start(wq_sb[:, 0:hw], wq[:, 0:hw])
            for k in range(nk):
                ld_x(k, 0, CHUNK, q3[k % 3])
            nc.scalar.dma_start(wk_sb[:, 0:hw], wk[:, 0:hw])
            nc.gpsimd.dma_start(wv_sb[:], wv[:])
            nc.gpsimd.dma_start(mask_sb[:], mask_c[:])
            nc.scalar.dma_start(wq_sb[:, hw:2 * hw], wq[:, hw:2 * hw])
            nc.scalar.dma_start(wk_sb[:, hw:2 * hw], wk[:, hw:2 * hw])
            for k in range(nk):
                ld_x(k, CHUNK, min(2 * CHUNK, seq), qs[k % 2])
            nc.gpsimd.dma_start(wo_sb[:], wo[:])
            for k in range(nk):
                ld_x(k, 2 * CHUNK, seq, qs[k % 2])

            def wsl(w, k, a, b):
                return w[:, k * HS + a:k * HS + b]

            def wsl_pm(w, pair, k):
                # pair-major packed wq/wk: [p, (pair*nk + k)*128 + j]
                return w[:, (pair * nk + k) * P:(pair * nk + k + 1) * P]

            # persistent QKV results
            qt_sb = [sb.tile([P, seq], BF16, tag=f"qt{p}", name=f"qt{p}")
                     for p in range(2)]
            kt_sb = [sb.tile([P, seq], BF16, tag=f"kt{p}", name=f"kt{p}")
                     for p in range(2)]
            v_sb = [sb.tile([P, HPC * DHA], BF16, tag=f"v{j}", name=f"v{j}")
                    for j in range(njt)]
            ot_sb = [sb.tile([P, seq], BF16, tag=f"ot{p}", name=f"ot{p}")
                     for p in range(2)]
            # ones columns of the augmented V are set once up front
            for jt in range(njt):
                nc.vector.memset(
                    v_sb[jt].rearrange("p (h d) -> p h d", h=HPC)[:, :,
                                                                  DH:DHA],
                    1.0)

            # ---- work generators: one ~512-row matmul per yield ----
            def g_kt(pair, ch):
                pt = ps.tile([P, CHUNK], F32, tag="misc",
                             name=f"ktps{pair}_{ch}", bufs=2)
                for k in range(nk):
                    nc.tensor.matmul(
                        pt[:], lhsT=wsl_pm(wk_sb, pair, k),
                        rhs=xt[k][:, ch * CHUNK:(ch + 1) * CHUNK],
                        start=(k == 0), stop=(k == nk - 1))
                    if k < nk - 1:
                        yield
                nc.vector.tensor_copy(
                    kt_sb[pair][:, ch * CHUNK:(ch + 1) * CHUNK], pt[:])
                yield

            def g_qt(pair, ch):
                pt = ps.tile([P, CHUNK], F32, tag="misc",
                             name=f"qps{pair}_{ch}", bufs=2)
                for k in range(nk):
                    nc.tensor.matmul(
                        pt[:], lhsT=wsl_pm(wq_sb, pair, k),
                        rhs=xt[k][:, ch * CHUNK:(ch + 1) * CHUNK],
                        start=(k == 0), stop=(k == nk - 1))
                    if k < nk - 1:
                        yield
                nc.vector.tensor_copy(
                    qt_sb[pair][:, ch * CHUNK:(ch + 1) * CHUNK], pt[:])
                yield

            def g_v(jt):
                # N=256 matmuls: two per yield to keep step granularity even
                pt = ps.tile([P, HS], F32, tag="misc",
                             name=f"vps{jt}", bufs=2)
                for k in range(nk):
                    nc.tensor.matmul(
                        pt[:], lhsT=xt[k][:, jt * P:(jt + 1) * P],
                        rhs=wsl(wv_sb, k, 0, HS),
                        start=(k == 0), stop=(k == nk - 1))
                    if k % 2 == 1 and k < nk - 1:
                        yield
                nc.vector.tensor_copy(
                    v_sb[jt].rearrange("p (h d) -> p h d", h=HPC)[:, :, 0:DH],
                    pt.rearrange("p (h d) -> p h d", h=HPC))
                yield

            def g_proj(ci, m, slices, korder, op_box=None, evac=True,
                       loads=None):
                # transposed output block: outT[m*128:(m+1)*128, chunk ci]
                # = Wo[:, m-slice].T @ attT[:, chunk] over the k-tiles in
                # `korder` (a partial pass parks its psum in op_box).
                # `loads` issues the gathered-tile DMAs as a prologue: the
                # generator only runs once its AllGather has landed, so the
                # DMAs don't sem-block their queues.
                c0 = ci * CHUNK
                if loads:
                    # pin the gathered-tile DMAs behind the last exp: the
                    # tile scheduler's collective cost model knows nothing
                    # of the ~55us CC-stream init, and without a real edge
                    # it places these sem-blocking DMAs mid-body on the
                    # scalar queue, stalling the exp stream for ~20us
                    for q, dst, src in loads:
                        di = q.dma_start(dst, src)
                        if last_exp[0] is not None:
                            add_dep_helper(di.ins, last_exp[0],
                                           reason="ag tile load gated to drain")
                    yield
                first = op_box is None or op_box.get(m) is None
                if first:
                    op_ps = ps.tile([P, CHUNK], F32, tag="misc",
                                    name=f"op{ci}_{m}", bufs=2)
                else:
                    op_ps = op_box.pop(m)
                for n, k in enumerate(korder):
                    ag_t, coff = slices[k]
                    nc.tensor.matmul(
                        op_ps[:],
                        lhsT=wsl(wo_sb, k, m * P, (m + 1) * P),
                        rhs=ag_t[:, coff:coff + CHUNK],
                        start=(first and n == 0),
                        stop=(evac and n == len(korder) - 1))
                    if n < len(korder) - 1:
                        yield
                if not evac:
                    op_box[m] = op_ps
                    yield
                    return
                o_sb = sb.tile([P, CHUNK], BF16, tag="osb",
                               name=f"o{ci}_{m}", bufs=2)
                ei = nc.vector.tensor_copy(o_sb[:], op_ps[:])
                if last_norm[0] is not None:
                    # keep the last chunk's normalize at the head of the
                    # drain's DVE queue so the final AllGather fires promptly
                    add_dep_helper(ei.ins, last_norm[0], sync=False,
                                   reason="drain evac after last normalize")
                oi = nc.sync.dma_start(
                    outT[m * P:(m + 1) * P, c0:c0 + CHUNK], o_sb[:])
                if last_bounce[0] is not None:
                    add_dep_helper(oi.ins, last_bounce[0], sync=False,
                                   reason="drain out after last bounce")
                yield

            # work:      [deadline, gen]  — ungated QKV work, FIFO
            # work_late: [earliest, deadline, gen] — AllGather-gated
            #            projections, FIFO; popped only once pos>=earliest
            work = deque()
            work_late = deque()
            INF = 1e9

            def drain_until(pos):
                # everything the upcoming S/PV slots depend on must already
                # be emitted on the in-order queues
                while work and work[0][0] <= pos:
                    for _ in work[0][1]:
                        pass
                    work.popleft()
                while work_late and work_late[0][1] <= pos:
                    for _ in work_late[0][2]:
                        pass
                    work_late.popleft()

            def step_work(n, pos, slack=0.0):
                # pop n steps; keep popping (up to 2 extra) while the head's
                # deadline is within `slack` — spreads what would otherwise
                # be a forced burst of matmuls at a pass boundary
                popped = 0
                while popped < n + 2:
                    if work:
                        if popped >= n and work[0][0] > pos + slack:
                            break
                        try:
                            next(work[0][1])
                            popped += 1
                        except StopIteration:
                            work.popleft()
                    elif work_late and work_late[0][0] <= pos:
                        if popped >= n:
                            break
                        try:
                            next(work_late[0][2])
                            popped += 1
                        except StopIteration:
                            work_late.popleft()
                    else:
                        break

            # ---- upfront: chunk-0 pass A deps emitted inline ----
            for _ in g_qt(0, 0):
                pass
            for _ in g_v(0):
                pass
            for _ in g_kt(0, 0):
                pass
            work.append([0.125, g_v(1)])
            work.append([0.25, g_v(2)])
            work.append([0.375, g_v(3)])
            work.append([0.5, g_qt(1, 0)])
            work.append([0.5, g_kt(1, 0)])

            # ---- attention chunks ----
            last_parts = {}
            last_loads = {}
            last_exp = [None]
            last_cc = [None]
            last_norm = [None]
            last_bounce = [None]
            for ci in range(nch):
                jt_end = jpc * (ci + 1)
                c0 = ci * CHUNK
                last = ci == nch - 1

                if ci + 1 < nch:
                    njt_end = jpc * (ci + 2)
                    work.append([ci + 0.85, g_qt(0, ci + 1)])
                    work.append([ci + 0.92, g_kt(0, ci + 1)])
                    for jt in range(jpc * (ci + 1), jpc * (ci + 2)):
                        # consumed by PV(jt) in pass A of chunk ci+1
                        work.append(
                            [ci + 1 + jt / (2 * njt_end), g_v(jt)])
                    work.append([ci + 1.42, g_qt(1, ci + 1)])
                    work.append([ci + 1.47, g_kt(1, ci + 1)])

                # one bounce buffer per chunk: with bufs=2 a later chunk's
                # bounce DMA WAR-waits for an earlier AllGather to consume
                # its buffer, sem-blocking the sync queue for tens of us
                binf = None if last else dram.tile(
                    [2 * P, CHUNK], BF16, tag="binf", name=f"binf{ci}",
                    bufs=4)

                ot_ps = {}
                pend = [None]

                def do_pass_end(hp, ci=ci, c0=c0, last=last, binf=binf,
                                ot_ps=ot_ps):
                    # softmax normalize: rcp of each head's sum row (staged
                    # to SBUF), gpsimd broadcast, then one mul per head
                    # reading the O^T psum directly (no staging copy)
                    for h2 in range(2):
                        # reciprocal must read from SBUF: straight from the
                        # PSUM sum row it returns garbage on hardware (sim
                        # accepts it)
                        srow = sb.tile([1, CHUNK], F32, tag=f"sr{h2}",
                                       name=f"sr{ci}_{hp}_{h2}", bufs=2)
                        nc.vector.tensor_copy(srow[:],
                                              ot_ps[hp][h2][DH:DHA, :])
                        rcp = sb.tile([1, CHUNK], F32, tag=f"rcp{h2}",
                                      name=f"rcp{ci}_{hp}_{h2}", bufs=2)
                        nc.vector.reciprocal_approx_fast(rcp[:], srow[:])
                        bc_sb = sb.tile([DH, CHUNK], F32, tag=f"bc{h2}",
                                        name=f"bc{ci}_{hp}_{h2}", bufs=2)
                        nc.gpsimd.partition_broadcast(bc_sb[:], rcp[:],
                                                      channels=DH)
                        mi = nc.vector.tensor_mul(
                            ot_sb[hp][h2 * DH:(h2 + 1) * DH, c0:c0 + CHUNK],
                            ot_ps[hp][h2][0:DH, :],
                            bc_sb[:])
                        if last and hp == 1:
                            last_norm[0] = mi.ins
                    if last:
                        tiles, loads = emit_ag_pair(ci, hp)
                        last_parts.update(tiles)
                        last_loads[hp] = loads
                    else:
                        nc.sync.dma_start(
                            binf[hp * P:(hp + 1) * P, :],
                            ot_sb[hp][:, c0:c0 + CHUNK])

                def emit_ag_full(ci, bounce_in):
                    # one AllGather for both head pairs of chunk ci (256KB;
                    # rank-major rows land so gathered row-block k*128 is
                    # exactly attT k-tile k).  Tiles are allocated here but
                    # DMA'd from the proj generator's prologue, after the
                    # gather has landed.  ch0/ch1 tiles ride sync (popped
                    # only once landed); ch2 is drain-only and its loads on
                    # gpsimd/scalar may block harmlessly there.
                    bounce_out = dram.tile([grp * 2 * P, CHUNK], BF16,
                                           tag="boutf", name=f"boutf{ci}",
                                           bufs=4)
                    nc.gpsimd.collective_compute(
                        "AllGather", mybir.AluOpType.bypass,
                        replica_groups=groups,
                        ins=[bounce_in.opt()], outs=[bounce_out.opt()])
                    tiles = {}
                    loads = []
                    for k in range(nko):
                        t = sb.tile([P, CHUNK], BF16, tag=f"ag{k}",
                                    name=f"ag{ci}_{k}", bufs=3)
                        loads.append(
                            (nc.scalar, t[:],
                             bounce_out[k * P:(k + 1) * P, :]))
                        tiles[k] = t
                    return tiles, loads

                def emit_ag_pair(ci, pair):
                    # half AllGather (one head pair) of the last chunk —
                    # fired right after that pair's normalize.  Gathered
                    # k-tiles land at k = 2r+pair.  Pair B's bounce rides
                    # the then-idle scalar queue; tile DMAs are issued from
                    # the drain projections (pair B's on scalar/gpsimd,
                    # where sem-blocking is harmless).
                    c0 = ci * CHUNK
                    bounce_in = dram.tile([P, CHUNK], BF16,
                                          tag=f"binh{pair}",
                                          name=f"binh{ci}_{pair}", bufs=2)
                    bounce_out = dram.tile([grp * P, CHUNK], BF16,
                                           tag=f"bouth{pair}",
                                           name=f"bouth{ci}_{pair}", bufs=2)
                    bi = nc.sync.dma_start(
                        bounce_in[:], ot_sb[pair][:, c0:c0 + CHUNK])
                    if pair == 1:
                        last_bounce[0] = bi.ins
                    cc = nc.gpsimd.collective_compute(
                        "AllGather", mybir.AluOpType.bypass,
                        replica_groups=groups,
                        ins=[bounce_in.opt()], outs=[bounce_out.opt()])
                    last_cc[0] = cc.ins
                    tiles = {}
                    loads = []
                    lq = ([nc.sync, nc.scalar] if pair == 0
                          else [nc.scalar, nc.scalar])
                    for r in range(grp):
                        k = 2 * r + pair
                        t = sb.tile([P, CHUNK], BF16, tag=f"ag{k}",
                                    name=f"ag{ci}_{k}", bufs=3)
                        loads.append((lq[r % 2], t[:],
                                      bounce_out[r * P:(r + 1) * P, :]))
                        tiles[k] = t
                    return tiles, loads

                def flush(jt_end=jt_end, ot_ps=ot_ps, pend=pend):
                    if pend[0] is None:
                        return
                    hp, jt, es, rel = pend[0]
                    pend[0] = None
                    for h2 in range(2):
                        h = 2 * hp + h2
                        nc.tensor.matmul(
                            ot_ps[hp][h2][:, rel:CHUNK],
                            lhsT=v_sb[jt][:, h * DHA:(h + 1) * DHA],
                            rhs=es[:, h2 * CHUNK + rel:(h2 + 1) * CHUNK],
                            start=(jt == 0), stop=(jt == jt_end - 1))
                    if jt == jt_end - 1:
                        do_pass_end(hp)

                for hp in range(2):
                    for jt in range(jt_end):
                        pos = ci + (hp * jt_end + jt) / (2 * jt_end)
                        drain_until(pos)
                        if jt == 0:
                            ot_ps[hp] = [
                                ps.tile([DHA, CHUNK], F32, tag=f"ot{h2}",
                                        name=f"ot{ci}_{hp}_{h2}", bufs=1)
                                for h2 in range(2)]
                        rel = max(0, (jt - jpc * ci)) * P
                        diag = jt >= jpc * ci

                        s2 = ps.tile([P, 2 * CHUNK], F32, tag="s2",
                                     name=f"s{ci}_{hp}_{jt}", bufs=2)
                        es = sb.tile([P, 2 * CHUNK], BF16, tag="es",
                                     name=f"es{ci}_{hp}_{jt}", bufs=8)

                        for h2 in range(2):
                            # S^T tile = K_h @ Q_h^T (row-tiled, K=64; the
                            # two heads run concurrently in the PE array)
                            nc.tensor.matmul(
                                s2[:, h2 * CHUNK + rel:(h2 + 1) * CHUNK],
                                lhsT=kt_sb[hp][h2 * DH:(h2 + 1) * DH,
                                               jt * P:(jt + 1) * P],
                                rhs=qt_sb[hp][h2 * DH:(h2 + 1) * DH,
                                              c0 + rel:c0 + CHUNK],
                                start=True, stop=not diag,
                                tile_position=(h2 * DH, 0))
                        if diag:
                            # additive causal mask accumulated into the S
                            # psum: identity.T @ maskneg tile (PE-local, no
                            # DVE in the S->exp->PV chain)
                            for h2 in range(2):
                                nc.tensor.matmul(
                                    s2[:, h2 * CHUNK + rel:
                                       h2 * CHUNK + rel + P],
                                    lhsT=mask_sb[:, 2 * P:3 * P],
                                    rhs=mask_sb[:, h2 * P:(h2 + 1) * P],
                                    start=False, stop=True)
                        # one exp for both heads (both psum banks); flat AP
                        # off the diagonal (3D APs cost ~190ns extra on ACT)
                        if rel == 0:
                            ei = nc.scalar.activation(es[:], s2[:], ActFn.Exp)
                        else:
                            ei = nc.scalar.activation(
                                es.rearrange("p (t c) -> p t c",
                                             t=2)[:, :, rel:],
                                s2.rearrange("p (t c) -> p t c",
                                             t=2)[:, :, rel:],
                                ActFn.Exp)
                        last_exp[0] = ei.ins
                        # software pipeline: independent queued matmuls keep
                        # the PE streaming while exp(t) runs on ACT, then the
                        # previous slot's PV issues
                        step_work(2, pos, slack=2.5 / (2 * jt_end))
                        flush()
                        pend[0] = (hp, jt, es, rel)
                flush()

                if last:
                    agt = dict(last_parts)
                    last_parts = {}
                else:
                    agt, agloads = emit_ag_full(ci, binf)
                slices = [(agt[k], 0) for k in range(nko)]
                evens = [k for k in range(nko) if k % 2 == 0]
                odds = [k for k in range(nko) if k % 2 == 1]
                nm = HS // P
                if not last:
                    # all projections run in the final drain: the serial CC
                    # stream only comes up ~60-85us in (runtime barrier), so
                    # no gather lands before the body is nearly done — the
                    # drain's ~27us of proj matmuls overlaps the CC tail
                    for m in range(nm):
                        work_late.append(
                            [INF, INF,
                             g_proj(ci, m, slices, list(range(nko)),
                                    loads=agloads if m == 0 else None)])
                else:
                    # split each output block's projection: the even k-tiles
                    # (from the pair-A AllGather) run while pair-B flies
                    op_box = {}
                    for m in range(nm):
                        work_late.append(
                            [INF, INF,
                             g_proj(ci, m, slices, evens, op_box=op_box,
                                    evac=False,
                                    loads=last_loads[0] if m == 0 else None)])
                    for m in range(nm):
                        work_late.append(
                            [INF, INF,
                             g_proj(ci, m, slices, odds, op_box=op_box,
                                    loads=last_loads[1] if m == 0 else None)])

            # final drain: ungated leftovers first, then the AG-gated
            # projections in order — everything but the last pair-B
            # projections is ready PE work spanning the last AG's flight
            while work:
                for _ in work[0][1]:
                    pass
                work.popleft()
            while work_late:
                for _ in work_late[0][2]:
                    pass
                work_late.popleft()

    if compile:
        nc.compile()
    return nc


def make_in_maps(x, Wq, Wk, Wv, Wo, n_cores=N_CORES):
    import ml_dtypes
    bf16 = ml_dtypes.bfloat16
    scale = np.float32(DH ** -0.5)
    # additive band mask for the diagonal j-tile of S^T [j,i]: 0 where
    # j <= i, -30000 where j > i (duplicated side by side for both heads),
    # plus the identity used to matmul-accumulate it into the S psum
    mask_b = np.where(np.triu(np.ones((P, P), np.float32)) > 0,
                      0.0, MASKNEG).astype(np.float32)
    ident = np.eye(P, dtype=np.float32)
    mask2 = np.concatenate([mask_b, mask_b, ident], axis=1).astype(bf16)

    def pack(sl):
        # [ntk*128, HS] -> [128, ntk*HS]: row p holds k-tile blocks side by
        # side so the whole weight is one contiguous-row DMA
        ntk = sl.shape[0] // P
        return np.ascontiguousarray(
            sl.reshape(ntk, P, HS).transpose(1, 0, 2).reshape(P, ntk * HS)
        ).astype(bf16)

    def pack_pm(sl):
        # pair-major: [nk*128, 2*128] -> [128, 2*nk*128] with
        # out[p, (pair*nk + k)*128 + j] = sl[k*128 + p, pair*128 + j],
        # so each head-pair's weights are one contiguous half
        ntk = sl.shape[0] // P
        return np.ascontiguousarray(
            sl.reshape(ntk, P, 2, P).transpose(1, 2, 0, 3).reshape(
                P, 2 * ntk * P)
        ).astype(bf16)

    in_maps = []
    for c in range(n_cores):
        b, r = divmod(c, 4)
        hs = r * HS
        in_maps.append({
            "xT": np.ascontiguousarray(x[b].T).astype(bf16),
            "wq": pack_pm(Wq[:, hs:hs + HS] * scale),
            "wk": pack_pm(Wk[:, hs:hs + HS]),
            "wv": pack(Wv[:, hs:hs + HS]),
            "wo": pack(Wo[:, hs:hs + HS]),
            "mask_c": mask2,
        })
    return in_maps


def assemble_out(results, B, seq, n_cores=N_CORES):
    out = np.empty((B, seq, INNER), np.float32)
    for c in range(n_cores):
        b, r = divmod(c, 4)
        out[b][:, r * HS:(r + 1) * HS] = results[c]["outT"].T.astype(
            np.float32)
    return out


_NC_CACHE = {}


def kernel(x, Wq, Wk, Wv, Wo):
    from concourse import bass_utils
    x = np.asarray(x, np.float32)
    B, seq, dim = x.shape
    key = (seq, dim)
    if key not in _NC_CACHE:
        _NC_CACHE[key] = build_nc(seq=seq, dim=dim)
    nc = _NC_CACHE[key]
    in_maps = make_in_maps(x, np.asarray(Wq, np.float32),
                           np.asarray(Wk, np.float32),
                           np.asarray(Wv, np.float32),
                           np.asarray(Wo, np.float32))
    res = bass_utils.run_bass_kernel_spmd(
        nc, in_maps, core_ids=list(range(N_CORES)))
    return assemble_out(res.results, B, seq)
